# revision 49
# baseline (speedup 1.0000x reference)
"""Trainium2 Bass kernel for nn_AllGeomLoss (retrieval_knn).

Self-contained: takes FULL inputs, shards rows across 8 NeuronCores
internally (data-parallel, 512 rows/core), returns the full scalar output.

Per-core algorithm (everything on-device except the final 8-way scalar sum):
  - recon: partial sum of (outputs-targets)^2 over this core's 512 rows.
  - global latent covariance C accumulated on PE (bf16 inputs) fused with
    the column-sum via a ones column; pr = 0.01*tr(C)^2/||C||_F^2 exactly;
    aniso via lambda_max from 5 matrix squarings + Rayleigh quotient.
  - kNN: biased negated-distance rows d'[i,j] = 2 raw_i.raw_j - |raw_j|^2
    (fp16 PSUM; +512 bias added later preserves ranking) via one PE matmul
    per 512-col chunk.  Per chunk max8 + max_index give top-8 values and
    indices; global candidate top-26 via index-embedding in the low 12
    mantissa bits of the 64 candidates, then 4 rounds of max8/
    match_replace8.  Rank 0 is self (d'[i,i] is the strict row max).
  - Neighbor gather: ONE batched gpsimd dma_gather per 128-row tile (3200
    int16 indices, 16-partition-wrapped and replicated), one SWDGE queue
    per tile so descriptor generation for the 4 tiles overlaps on the 8
    Q7 cores.  Index repack runs on PE (16-partition replicate matmuls) +
    one strided DVE cast-copy.
  - tsa: per-row top eigenvector of the 25-neighborhood covariance via one
    power iteration u = Yc^T(Yc v0), v0 = Y0 - Y1, computed for latent and
    raw sides JOINTLY on [128, 25, 128] tiles; tsa needs only
    (uz.ux)^2/(|uz|^2|ux|^2).
  - Emission order: all 4 tiles' selection+gather first, then cov/recon
    work (fills the gather window), then the 4 eig stages — so the
    in-order Vector queue never head-of-line blocks on a gather.
"""
import os
import numpy as np

B, D = 4096, 64
NCORES = 8
RPC = B // NCORES          # rows per core = 512
NT = RPC // 128            # 128-row tiles per core = 4
K = 25
SEL_CHUNK = 512            # selection chunk size
NCHUNK = B // SEL_CHUNK    # 8
KEPS = 1.0 / (B - 1 + 1e-8)

_CACHE = {}


def _apply_compiler_workarounds():
    # This container's walrus build rejects instructions carrying more than
    # one sync-wait (Drain at the kernel tail collects one wait per DMA
    # queue semaphore).  Collapse the HW/SW DGE round-robin to a single
    # semaphore lane and spread the tail-drain waits over one-wait nops.
    import concourse.tile_sem_assignment as _tsa
    import concourse.tile as _tile

    if not getattr(_tile.TileContext, "_drain_split_patched", False):
        _orig_dab = _tile.TileContext._drain_and_barrier

        def _drain_and_barrier_split(self, tick_clock, wait_clock):
            from concourse.vector_clock import ScopedClock, VectorClock
            gc = tick_clock.global_clock
            for p in range(_tsa.N_PROCS):
                if gc[p] > 0:
                    part = [0] * _tsa.N_PROCS
                    part[p] = gc[p]
                    nop = self.nc.sync.nop(nofuse=True)
                    wait_clock.add_sem_waits(
                        nop.ins, ScopedClock({None: VectorClock(part)}))
            self.nc.sync.drain()
            self.nc.all_engine_barrier()
            assert self.sems is not None
            popped = self.nc._tile_sem_poison_stack.pop()
            assert popped is self._sem_poison
            self.nc.clear_and_free_semaphores(
                list(self.sems.allocated().values()))
            self.nc.all_engine_barrier()

        _tile.TileContext._drain_and_barrier = _drain_and_barrier_split
        _tile.TileContext._drain_split_patched = True

    from concourse.bass import Bass as _Bass
    if not getattr(_Bass, "_json_wait_split_patched", False):
        _orig_to_json = _Bass.to_json_bytes

        def _to_json_split_waits(self, *a, **kw):
            import json as _json
            raw = _orig_to_json(self, *a, **kw)
            m = _json.loads(raw)
            changed = False
            for f in m.get("functions", []):
                for blk in f.get("blocks", []):
                    insts = blk.get("instructions")
                    if not insts:
                        continue
                    new = []
                    for ins in insts:
                        if ins.get("opcode") == "ISA" and \
                                ins.get("op_name") == "SeqAssert":
                            # This walrus build rejects SeqAssert encodings
                            # ("ISA wrong length"); our dynamic values are
                            # partition ids with statically-known range.
                            changed = True
                            ins = {
                                "debug": ins.get("debug", 0),
                                "engine": ins["engine"],
                                "ins": [],
                                "name": ins["name"],
                                "opcode": "NoOp",
                                "outs": [],
                                "sync_info": ins.get("sync_info") or
                                {"on_update": [], "on_wait": []},
                            }
                        si = ins.get("sync_info") or {}
                        ow = si.get("on_wait") or []
                        if len(ow) > 1:
                            changed = True
                            for j, w in enumerate(ow[:-1]):
                                new.append({
                                    "debug": ins.get("debug", 0),
                                    "engine": ins["engine"],
                                    "ins": [],
                                    "name": f"{ins['name']}_wsplit{j}",
                                    "opcode": "NoOp",
                                    "outs": [],
                                    "sync_info": {"on_update": [],
                                                  "on_wait": [w]},
                                })
                            si["on_wait"] = [ow[-1]]
                        new.append(ins)
                    blk["instructions"] = new
            if not changed:
                return raw
            return _json.dumps(m).encode()

        _Bass.to_json_bytes = _to_json_split_waits
        _Bass._json_wait_split_patched = True


def _build_bass(reps=1, phase=None, ablate=None, dbg=False):
    from concourse.bass import Bass
    from concourse import mybir
    from concourse import library_config
    from concourse.tile import TileContext
    from contextlib import ExitStack

    _apply_compiler_workarounds()

    f32 = mybir.dt.float32
    f16 = mybir.dt.float16
    bf16 = mybir.dt.bfloat16
    u32 = mybir.dt.uint32
    i32 = mybir.dt.int32
    i16 = mybir.dt.int16

    nc = Bass(trn_type="TRN2", enable_asserts=False, num_swdge_queues=4)

    outputs_l = nc.dram_tensor("outputs_l", [RPC, D], f32, kind="ExternalInput")
    targets_l = nc.dram_tensor("targets_l", [RPC, D], f32, kind="ExternalInput")
    raw_l = nc.dram_tensor("raw_l", [RPC, D], f32, kind="ExternalInput")
    latent = nc.dram_tensor("latent", [B, D], f32, kind="ExternalInput")
    raw = nc.dram_tensor("raw", [B, D], f32, kind="ExternalInput")
    ident_in = nc.dram_tensor("ident", [128, 128], f32, kind="ExternalInput")
    iota_in = nc.dram_tensor("iotac", [128, NCHUNK * 8], u32,
                             kind="ExternalInput")
    # bitmask constants as tensor_tensor operands: cols 0:64 = 0x7FFFF000
    # (embed mask), cols 64:89 = 0xFFF (index decode mask).  DVE
    # tensor_scalar can enter 2-port perf mode and then fully blocks
    # against active SWDGE descriptor generation (the gathers);
    # tensor_tensor never contends, so masks come in as tensors.
    mask_in = nc.dram_tensor("masku", [128, 64 + K], u32,
                             kind="ExternalInput")
    res = nc.dram_tensor("res", [1, 1], f32, kind="ExternalOutput")
    combD = nc.dram_tensor("combD", [B, 2 * D], bf16)
    if dbg:
        dbg_ch = nc.dram_tensor("dbg_ch", [128, SEL_CHUNK], f32,
                                kind="ExternalOutput")
        dbg_cv = nc.dram_tensor("dbg_cv", [128, NCHUNK * 8], f32,
                                kind="ExternalOutput")
        dbg_ci = nc.dram_tensor("dbg_ci", [128, NCHUNK * 8], i32,
                                kind="ExternalOutput")
        dbg_idx = nc.dram_tensor("dbg_idx", [128, K], i32,
                                 kind="ExternalOutput")
        dbg_comb = nc.dram_tensor("dbg_comb", [128, K * 2 * D], f32,
                                  kind="ExternalOutput")
        dbg_stats = nc.dram_tensor("dbg_stats", [128, 8], f32,
                                   kind="ExternalOutput")
        dbg_u = nc.dram_tensor("dbg_u", [128, 2 * D], f32,
                               kind="ExternalOutput")
        dbg_sv = nc.dram_tensor("dbg_sv", [128, K * 2], f32,
                                kind="ExternalOutput")

    A = mybir.AluOpType
    AX = mybir.AxisListType

    def pr(name):
        return reps if phase == name else 1

    with nc.allow_low_precision("bf16/fp16 distance/eig stages within tol"), \
            TileContext(nc) as tc, ExitStack() as ctx:

        const_p = ctx.enter_context(tc.tile_pool(name="const", bufs=1))
        chunk_p = ctx.enter_context(tc.tile_pool(name="chunk", bufs=3))
        sel_p = ctx.enter_context(tc.tile_pool(name="sel", bufs=4))
        eig_p = ctx.enter_context(tc.tile_pool(name="eig", bufs=2))
        cg_p = ctx.enter_context(tc.tile_pool(name="cgp", bufs=4))
        psum_p = ctx.enter_context(tc.tile_pool(name="psum", bufs=4, space="PSUM"))
        psS = ctx.enter_context(tc.tile_pool(name="psS", bufs=2, space="PSUM"))
        cov_p = ctx.enter_context(tc.tile_pool(name="covp", bufs=1, space="PSUM"))

        # ---- constants ----
        ident = const_p.tile([128, 128], f32)
        identb = const_p.tile([128, 128], bf16)
        stats = const_p.tile([128, 8], f32)
        ones64b = const_p.tile([64, 1], bf16)
        ones64f = const_p.tile([64, 1], f32)
        ones128 = const_p.tile([128, 1], f32)
        iota_off = const_p.tile([128, NCHUNK * 8], u32)
        masks = const_p.tile([128, 64 + K], u32)
        negK = const_p.tile([128, 1], f32)
        kepsc = const_p.tile([64, 1], f32)

        # dma_gather (InstDMAGatherAnt) lives in the 'mlp' gpsimd library;
        # iota was replaced with a host-supplied constant so no standard-
        # library op remains and one load at kernel start suffices.
        nc.gpsimd.load_library(library_config.mlp)
        nc.sync.dma_start(ident[:], ident_in[:])
        nc.sync.dma_start(iota_off[:], iota_in[:])
        nc.sync.dma_start(masks[:], mask_in[:])
        nc.vector.tensor_copy(identb[:], ident[:])
        nc.vector.memset(ones64b[:], 1.0)
        nc.vector.memset(ones64f[:], 1.0)
        nc.vector.memset(ones128[:], 1.0)
        nc.vector.memset(stats[:], 0.0)
        nc.vector.memset(negK[:], -1.0 / K)
        nc.vector.memset(kepsc[:], KEPS)
        # Rt[:, g, :] is the [128, 128] f32 stationary R_g with
        # R_g[c, q] = 1 iff c == g*16 + q%16.  matmul(lhsT=R_g, rhs=idxf)
        # replicates rows [16g, 16g+16) of idxf onto all 8 16-partition
        # groups — the dma_gather index buffer wants the 16-partition wrap
        # replicated into every Q7 core window (queue q reads partitions
        # [32q, 32q+32), so full replication covers all queues).
        Rt = const_p.tile([128, 8, 128], f32)
        for g in range(8):
            nc.vector.tensor_copy(
                Rt[:, g, :].rearrange("p (k q) -> p k q", k=8),
                ident[:, 16 * g:16 * (g + 1)].unsqueeze(1)
                .broadcast_to([128, 8, 16]))

        for _rep in range(reps if phase is None else 1):
            # ---- prep: two full-table loads (p-major: 8KB descriptors),
            # bf16 comb table [1 | latent | raw], X matrix ----
            # global row j = 32*p + t  (partition-major layout)
            raw_f = const_p.tile([128, 32, D], f32, tag="rawf")
            lat_f = const_p.tile([128, 32, D], f32, tag="latf")
            comb = const_p.tile([128, 32, 2 * D + 1], bf16, tag="comb")
            nc.sync.dma_start(raw_f[:],
                              raw[:].rearrange("(p t) d -> p t d", p=128))
            nc.sync.dma_start(lat_f[:],
                              latent[:].rearrange("(p t) d -> p t d", p=128))
            # recon + local-raw loads issued early so the DMA overlaps prep
            ob = const_p.tile([128, NT, 64], f32, tag="ob")
            tb = const_p.tile([128, NT, 64], f32, tag="tb")
            nc.sync.dma_start(ob[:],
                              outputs_l[:].rearrange("(p t) d -> p t d", t=NT))
            nc.sync.dma_start(tb[:],
                              targets_l[:].rearrange("(p t) d -> p t d", t=NT))
            rloc = const_p.tile([128, NT, 64], f32, tag="rloc")
            nc.sync.dma_start(rloc[:],
                              raw_l[:].rearrange("(p t) d -> p t d", t=NT))

            nc.vector.memset(comb[:, :, 0:1], 1.0)
            nc.vector.tensor_copy(comb[:, :, 1:D + 1], lat_f[:])
            nc.vector.tensor_copy(comb[:, :, D + 1:2 * D + 1], raw_f[:])
            # combined bf16 table to DRAM (for the gathers); row j = 32p+t
            nc.sync.dma_start(
                combD[:].rearrange("(p t) c -> p t c", p=128, t=32),
                comb[:, :, 1:2 * D + 1])
            rlocb = const_p.tile([128, NT, 64], bf16, tag="rlocb")
            nc.vector.tensor_copy(rlocb[:], rloc[:])

            # ---- X = [rawT (64 rows); 512-sq] [65, 4096] bf16 ----
            # the +512 embed bias is folded into the bias row (ACT bias on
            # the sq write): the DVE tensor_scalar add it replaces blocks
            # against active SWDGE descriptor generation.
            # transpose of comb tile t gives raw rows {32p+t} -> X cols 32p+t
            X = const_p.tile([65, B], bf16)
            # Wb stationaries [65, 128]: rows 0:64 = 2*rawT_local (per
            # tile), row 64 = 1 (pairs 512-sq).  The constant row is
            # memset here, before any gather runs.
            Wbs = []
            for t in range(NT):
                Wb = sel_p.tile([65, 128], bf16, tag=f"Wb{t}")
                nc.vector.memset(Wb[64:65, :], 1.0)
                Wbs.append(Wb)
            Xv = X[0:64, :].rearrange("q (p t) -> q t p", t=32)
            for g in range(8):
                pT_ps = psS.tile([64, 4, 128], bf16, tag="s", space="PSUM")
                for u in range(4):
                    nc.tensor.transpose(out=pT_ps[:, u, :],
                                        in_=comb[:, 4 * g + u, D + 1:2 * D + 1],
                                        identity=identb[:])
                # DVE copy beats ACT for these [64, 512] bf16 moves and
                # runs pre-gather (no SWDGE 2-port conflict possible).
                nc.vector.tensor_copy(Xv[:, 4 * g:4 * g + 4, :], pT_ps[:])
            for c in range(NCHUNK):
                cs = slice(c * SEL_CHUNK, (c + 1) * SEL_CHUNK)
                sq_t = chunk_p.tile([64, SEL_CHUNK], bf16, tag="sqt")
                nc.vector.tensor_mul(sq_t[:], X[0:64, cs], X[0:64, cs])
                sq_ps = psS.tile([1, SEL_CHUNK], f32, tag="s", space="PSUM")
                nc.tensor.matmul(out=sq_ps[:], lhsT=ones64b[:], rhs=sq_t[:],
                                 start=True, stop=True)
                # X row 64 = 512 - |r_j|^2 (bias bakes the embed offset in)
                nc.scalar.activation(X[64:65, cs], sq_ps[:],
                                     mybir.ActivationFunctionType.Copy,
                                     bias=512.0, scale=-1.0)

            # ---- per 128-row tile: selection + batched gather ----
            def emit_selgather(t):
                Wb = Wbs[t]
                rT2_ps = psS.tile([64, 128], bf16, tag="s", space="PSUM")
                nc.tensor.transpose(out=rT2_ps[:], in_=rlocb[:, t, :],
                                    identity=identb[:])
                nc.scalar.mul(Wb[0:64, :], rT2_ps[:], 2.0)

                cand_v = sel_p.tile([128, NCHUNK * 8], f32, tag="cand_v")
                cand_i = sel_p.tile([128, NCHUNK * 8], u32, tag="cand_i")
                for c in range(NCHUNK):
                    ps_d = psum_p.tile([128, SEL_CHUNK], f32, tag="dist",
                                       space="PSUM")
                    for _dr in range(pr("dist")):
                        nc.tensor.matmul(
                            out=ps_d[:], lhsT=Wb[:],
                            rhs=X[:, c * SEL_CHUNK:(c + 1) * SEL_CHUNK],
                            start=True, stop=True)
                    if dbg and t == 0 and c == 0:
                        chf = chunk_p.tile([128, SEL_CHUNK], f32, tag="chf")
                        nc.vector.tensor_copy(chf[:], ps_d[:])
                        nc.sync.dma_start(dbg_ch[:], chf[:])
                    for _sr in range(pr("sel")):
                        nc.vector.max(cand_v[:, c * 8:(c + 1) * 8], ps_d[:])
                        nc.vector.max_index(cand_i[:, c * 8:(c + 1) * 8],
                                            cand_v[:, c * 8:(c + 1) * 8],
                                            ps_d[:])
                # embed global index into low 12 mantissa bits of the 64
                # candidates (+512 bias first: values ~[350,620),
                # quantum <= 0.25)
                emb = sel_p.tile([128, NCHUNK * 8], u32, tag="emb")
                top32 = sel_p.tile([128, 32], u32, tag="top32")
                idx32 = sel_p.tile([128, K], i32, tag="idx32")
                for _sr in range(pr("sel")):
                    nc.vector.tensor_tensor(out=cand_i[:], in0=cand_i[:],
                                            in1=iota_off[:], op=A.add)
                    # cand_v already carries the +512 bias from the matmul
                    nc.vector.tensor_tensor(out=emb[:],
                                            in0=cand_v[:].bitcast(u32),
                                            in1=masks[:, 0:64],
                                            op=A.bitwise_and)
                    nc.vector.tensor_tensor(out=emb[:], in0=emb[:],
                                            in1=cand_i[:], op=A.bitwise_or)
                    for r in range(4):
                        nc.vector.max(top32[:, r * 8:(r + 1) * 8].bitcast(f32),
                                      emb[:].bitcast(f32))
                        if r < 3:
                            nc.vector.match_replace(
                                out=emb[:].bitcast(f32),
                                in_to_replace=top32[:, r * 8:(r + 1) * 8]
                                .bitcast(f32),
                                in_values=emb[:].bitcast(f32), imm_value=0.0)
                    # decode 25 neighbor indices (drop rank 0 = self)
                    nc.vector.tensor_tensor(out=idx32[:].bitcast(u32),
                                            in0=top32[:, 1:1 + K],
                                            in1=masks[:, 64:64 + K],
                                            op=A.bitwise_and)

                if dbg and t == 0:
                    nc.sync.dma_start(dbg_cv[:], cand_v[:])
                    nc.sync.dma_start(dbg_ci[:], cand_i[:].bitcast(i32))
                    nc.sync.dma_start(dbg_idx[:], idx32[:])
                comb_g = cg_p.tile([128, K, 2 * D], bf16, tag="comb_g")
                if ablate == "gather":
                    nc.vector.memset(comb_g[:], 1.0)
                else:
                    # ONE batched dma_gather per tile.  dma_gather reads
                    # int16 indices wrapped in 16 partitions (linear
                    # n = s*16 + p), replicated to every 16-partition
                    # group, and writes gathered row n to
                    # dst[n % 128, n // 128, :]: with n = a*128 + i the
                    # index at [p, 8a + g] must be idx32[g*16 + p, a].
                    # E16 matmuls replicate rows 16g..16g+16 across all
                    # partition groups; the strided cast-copy interleaves
                    # (g, a) -> column a*8 + g.
                    # both casts run on the Scalar engine so the gather's
                    # input chain never sits behind eig work in the
                    # in-order Vector queue.
                    idxf = sel_p.tile([128, K], f32, tag="idxf")
                    nc.scalar.copy(idxf[:], idx32[:])
                    rep_ps = psS.tile([128, 8, K], f32, tag="s", space="PSUM")
                    for g in range(8):
                        nc.tensor.matmul(out=rep_ps[:, g, :],
                                         lhsT=Rt[:, g, :], rhs=idxf[:],
                                         start=True, stop=True)
                    idxs16 = sel_p.tile([128, 8 * K], i16, tag="idxs16")
                    nc.scalar.copy(
                        idxs16[:].rearrange("p (a g) -> p g a", g=8),
                        rep_ps[:])
                    for _gr in range(pr("gather")):
                        # single_packet=False: coalescing 3200 descs into
                        # one packet per engine exceeds the <=64-descriptor
                        # packet ceiling and wedges the device.  One SWDGE
                        # queue per tile: descriptor generation runs on a
                        # different Q7 core pair per queue and overlaps.
                        nc.gpsimd.dma_gather(
                            out_ap=comb_g[:], in_ap=combD[:],
                            idxs_ap=idxs16[:], num_idxs=128 * K,
                            num_idxs_reg=128 * K, elem_size=2 * D,
                            single_packet=False, queue_num=t)

                if dbg and t == 0:
                    cgf = eig_p.tile([128, K * 2 * D], f32, tag="cgf")
                    nc.vector.tensor_copy(
                        cgf[:], comb_g[:].rearrange("p k c -> p (k c)"))
                    nc.sync.dma_start(dbg_comb[:], cgf[:])
                return comb_g, idx32

            # ---- eig: both sides jointly; one power iteration ----
            eig_count = [0]

            def emit_eig(comb_g, gate_b):
                first_eig = eig_count[0] == 0
                eig_count[0] += 1
                for _er in range(pr("eig")):
                    # gate_b is all-zeros, produced after the LAST tile's
                    # index decode: a real dependency on the FIRST op of
                    # the eig chain keeps every eig Vector op behind all
                    # selection/decode Vector ops in any schedule (the
                    # in-order Vector queue would otherwise head-of-line
                    # block later selections — and thus the gathers — on
                    # this eig's gather data).  All other eig ops depend
                    # on v0 or its descendants.
                    v0 = eig_p.tile([128, 2 * D], bf16, tag="v0")
                    nc.vector.tensor_tensor(
                        out=v0[:], in0=comb_g[:, 0, :],
                        in1=gate_b[:].broadcast_to([128, 2 * D]), op=A.add)
                    nc.vector.tensor_sub(v0[:], v0[:], comb_g[:, 1, :])
                    t1 = eig_p.tile([128, K, 2 * D], bf16, tag="t1")
                    nc.vector.tensor_tensor(
                        out=t1[:], in0=comb_g[:],
                        in1=v0[:].unsqueeze(1).broadcast_to([128, K, 2 * D]),
                        op=A.mult)
                    # per-(neighbor, side) dots: reduce innermost 64
                    s_v = eig_p.tile([128, K, 2], f32, tag="sv")
                    nc.vector.tensor_reduce(
                        out=s_v[:],
                        in_=t1[:].rearrange("p k (s d) -> p k s d", s=2),
                        axis=AX.X, op=A.add)
                    ssum = eig_p.tile([128, 2], f32, tag="ssum")
                    nc.vector.tensor_reduce(
                        out=ssum[:], in_=s_v[:].rearrange("p k s -> p s k"),
                        axis=AX.X, op=A.add)
                    # center: s = s - mean_k(s), via two tensor_tensor ops
                    # (scalar_tensor_tensor may enter the 2-port perf mode
                    # that blocks against active SWDGE generation)
                    nc.vector.tensor_tensor(
                        out=ssum[:], in0=ssum[:],
                        in1=negK[:].broadcast_to([128, 2]), op=A.mult)
                    nc.vector.tensor_tensor(
                        out=s_v[:],
                        in0=ssum[:].unsqueeze(1).broadcast_to([128, K, 2]),
                        in1=s_v[:], op=A.add)
                    # t2 = Y * s_bc, then tree-reduce over k
                    t2 = eig_p.tile([128, K + 7, 2 * D], bf16, tag="t2")
                    nc.vector.tensor_tensor(
                        out=t2[:, 0:K, :].rearrange("p k (s d) -> p k s d", s=2),
                        in0=comb_g[:].rearrange("p k (s d) -> p k s d", s=2),
                        in1=s_v[:].unsqueeze(3).broadcast_to([128, K, 2, D]),
                        op=A.mult)
                    n = K
                    while n > 1:
                        h = n // 2
                        nc.vector.tensor_add(t2[:, 0:h, :], t2[:, 0:h, :],
                                             t2[:, h:2 * h, :])
                        if n % 2:
                            # move via add-zero: tensor_copy can enter the
                            # 2-port mode that blocks against SWDGE
                            nc.vector.tensor_tensor(
                                out=t2[:, h:h + 1, :],
                                in0=t2[:, n - 1:n, :],
                                in1=gate_b[:].unsqueeze(1)
                                .broadcast_to([128, 1, 2 * D]), op=A.add)
                            n = h + 1
                        else:
                            n = h
                    if dbg and first_eig:
                        svf = eig_p.tile([128, K * 2], f32, tag="svf")
                        nc.vector.tensor_copy(
                            svf[:], s_v[:].rearrange("p k s -> p (k s)"))
                        nc.sync.dma_start(dbg_sv[:], svf[:])
                    # overlap stats: q = (uz.ux)^2 / (|uz|^2 |ux|^2)
                    u = t2[:, 0, :]
                    u2 = eig_p.tile([128, 2 * D], f32, tag="u2")
                    nc.vector.tensor_mul(u2[:], u, u)
                    nn_v = eig_p.tile([128, 2], f32, tag="nn")
                    nc.vector.tensor_reduce(
                        out=nn_v[:], in_=u2[:].rearrange("p (s d) -> p s d", s=2),
                        axis=AX.X, op=A.add)
                    cr = eig_p.tile([128, D], f32, tag="cr")
                    nc.vector.tensor_mul(cr[:], u[:, 0:D], u[:, D:2 * D])
                    dzx = eig_p.tile([128, 2], f32, tag="dzx")
                    nc.vector.tensor_reduce(out=dzx[:, 0:1], in_=cr[:],
                                            axis=AX.X, op=A.add)
                    nc.vector.tensor_mul(dzx[:, 1:2], nn_v[:, 0:1], nn_v[:, 1:2])
                    # +1 guards div-by-zero for degenerate rows (duplicate
                    # neighbors from exact distance ties); den is ~1e10
                    # normally so the bias is negligible.
                    nc.vector.tensor_add(dzx[:, 1:2], dzx[:, 1:2],
                                         ones128[:, 0:1])
                    nc.vector.reciprocal(dzx[:, 1:2], dzx[:, 1:2])
                    nc.vector.tensor_mul(dzx[:, 0:1], dzx[:, 0:1], dzx[:, 0:1])
                    nc.vector.tensor_mul(dzx[:, 0:1], dzx[:, 0:1], dzx[:, 1:2])
                    nc.vector.tensor_add(stats[:, 1:2], stats[:, 1:2],
                                         dzx[:, 0:1])
                    if dbg and first_eig:
                        uf = eig_p.tile([128, 2 * D], f32, tag="uf")
                        nc.vector.tensor_copy(uf[:], u)
                        nc.sync.dma_start(dbg_u[:], uf[:])

            # all 4 selections+gathers first: the gathers start as early as
            # possible and run on their own SWDGE queues while the Vector
            # engine keeps busy with the following selections + cov/recon.
            sel_out = [emit_selgather(t) for t in range(NT)]
            cgs = [cg for cg, _ in sel_out]
            # zero gate derived from the LAST tile's decoded indices
            # (x - x == 0; tensor_tensor form never contends with SWDGE)
            gate_b = const_p.tile([128, 1], bf16, tag="gate")
            nc.vector.tensor_tensor(out=gate_b[:], in0=sel_out[-1][1][:, 0:1],
                                    in1=sel_out[-1][1][:, 0:1],
                                    op=A.subtract)

            # ---- global latent covariance on PE (bf16), fused with the
            # column-sum via the ones column: out [64, 65] ----
            cov_ps = cov_p.tile([64, 65], f32, space="PSUM")
            for t in range(32):
                nc.tensor.matmul(out=cov_ps[:], lhsT=comb[:, t, 1:D + 1],
                                 rhs=comb[:, t, 0:D + 1],
                                 start=(t == 0), stop=(t == 31))

            # ---- cov postprocessing: C, trC, trC2, lambda_max ----
            cov_s = const_p.tile([64, 65], f32, tag="covs")
            nc.scalar.copy(cov_s[:], cov_ps[:])
            # s as a row: s_row[0, f] = s[f] via lhsT = s_col
            srow_ps = psS.tile([1, 64], f32, tag="s", space="PSUM")
            nc.tensor.matmul(out=srow_ps[:], lhsT=cov_s[:, 0:1],
                             rhs=ident[0:64, 0:64], start=True, stop=True)
            s_row = const_p.tile([1, 64], f32, tag="srow")
            nc.scalar.copy(s_row[:], srow_ps[:])
            ssT_ps = psS.tile([64, 64], f32, tag="s", space="PSUM")
            nc.tensor.matmul(out=ssT_ps[:], lhsT=s_row[:], rhs=s_row[:],
                             start=True, stop=True)
            sst_s = const_p.tile([64, 64], f32, tag="sst")
            nc.scalar.mul(sst_s[:], ssT_ps[:], KEPS / B)
            C_s = const_p.tile([64, 64], f32, tag="Cs")
            # C = cov*KEPS - ssT*(KEPS/B), via tensor_tensor forms only
            # (this block overlaps the gathers)
            nc.vector.tensor_tensor(out=C_s[:], in0=cov_s[:, 1:D + 1],
                                    in1=kepsc[:].broadcast_to([64, 64]),
                                    op=A.mult)
            nc.vector.tensor_sub(C_s[:], C_s[:], sst_s[:])
            diag_scr = const_p.tile([64, 64], f32, tag="dscr")
            nc.vector.tensor_mul(diag_scr[:], C_s[:], ident[0:64, 0:64])
            nc.vector.tensor_reduce(out=stats[0:64, 2:3], in_=diag_scr[:],
                                    axis=AX.X, op=A.add)
            fro_scr = const_p.tile([64, 64], f32, tag="fscr")
            nc.vector.tensor_mul(fro_scr[:], C_s[:], C_s[:])
            nc.vector.tensor_reduce(out=stats[0:64, 3:4], in_=fro_scr[:],
                                    axis=AX.X, op=A.add)
            # 5 squarings: M = C^32, then Rayleigh via w = M.1
            M_prev = C_s
            for sqi in range(5):
                m_ps = psS.tile([64, 64], f32, tag="s", space="PSUM")
                nc.tensor.matmul(out=m_ps[:], lhsT=M_prev[:], rhs=M_prev[:],
                                 start=True, stop=True)
                M_new = const_p.tile([64, 64], f32, tag=f"m{sqi}")
                nc.scalar.copy(M_new[:], m_ps[:])
                M_prev = M_new
            w_ps = psS.tile([64, 1], f32, tag="s", space="PSUM")
            nc.tensor.matmul(out=w_ps[:], lhsT=M_prev[:], rhs=ones64f[:],
                             start=True, stop=True)
            w_s = const_p.tile([64, 1], f32, tag="ws")
            nc.scalar.copy(w_s[:], w_ps[:])
            r_ps = psS.tile([64, 1], f32, tag="s", space="PSUM")
            nc.tensor.matmul(out=r_ps[:], lhsT=C_s[:], rhs=w_s[:],
                             start=True, stop=True)
            nc.vector.tensor_mul(stats[0:64, 4:5], w_s[:], r_ps[:])
            nc.vector.tensor_mul(stats[0:64, 5:6], w_s[:], w_s[:])

            # ---- recon over this core's 512-row slice ----
            dif = const_p.tile([128, NT, 64], f32, tag="dif")
            nc.vector.tensor_sub(dif[:], ob[:], tb[:])
            nc.vector.tensor_mul(dif[:], dif[:], dif[:])
            nc.vector.tensor_reduce(out=stats[:, 0:1], in_=dif[:],
                                    axis=AX.XY, op=A.add)

            # ---- eig stages, in gather-completion order ----
            if ablate != "eig":
                for t in range(NT):
                    emit_eig(cgs[t], gate_b)

        if dbg:
            nc.sync.dma_start(dbg_stats[:], stats[:])
        # ---- final scalar assembly ----
        fin_ps = psS.tile([1, 8], f32, tag="s", space="PSUM")
        nc.tensor.matmul(out=fin_ps[:], lhsT=ones128[:], rhs=stats[:],
                         start=True, stop=True)
        fin = const_p.tile([1, 8], f32, tag="fin")
        nc.scalar.copy(fin[:], fin_ps[:])
        sc = const_p.tile([1, 8], f32, tag="sc")
        res_s = const_p.tile([1, 1], f32, tag="ress")
        nc.vector.reciprocal(sc[:, 0:1], fin[:, 3:4])          # 1/trC2
        nc.vector.reciprocal(sc[:, 1:2], fin[:, 5:6])          # 1/(w.w)
        nc.vector.reciprocal(sc[:, 2:3], fin[:, 2:3])          # 1/trC
        nc.vector.tensor_mul(sc[:, 3:4], fin[:, 2:3], fin[:, 2:3])
        nc.vector.tensor_mul(sc[:, 3:4], sc[:, 3:4], sc[:, 0:1])   # pr ratio
        nc.vector.tensor_mul(sc[:, 4:5], fin[:, 4:5], sc[:, 1:2])  # lambda
        nc.vector.tensor_mul(sc[:, 4:5], sc[:, 4:5], sc[:, 2:3])   # lam/trC
        # S = f0/262144 + 0.02625 - (0.2/4096) f1 + 0.00125 pr - 0.00125 q
        nc.vector.tensor_scalar(res_s[:], fin[:, 0:1], 1.0 / (B * D), 0.02625,
                                op0=A.mult, op1=A.add)
        nc.vector.scalar_tensor_tensor(out=res_s[:], in0=fin[:, 1:2],
                                       scalar=-0.2 / B, in1=res_s[:],
                                       op0=A.mult, op1=A.add)
        nc.vector.scalar_tensor_tensor(out=res_s[:], in0=sc[:, 3:4],
                                       scalar=0.00125, in1=res_s[:],
                                       op0=A.mult, op1=A.add)
        nc.vector.scalar_tensor_tensor(out=res_s[:], in0=sc[:, 4:5],
                                       scalar=-0.00125, in1=res_s[:],
                                       op0=A.mult, op1=A.add)
        nc.sync.dma_start(res[:], res_s[:])

    # Raw Bass skips Bacc.compile(); fill in the ISA encoding bytes for
    # extended-inst ISA subclasses (PseudoReloadLibraryIndex) — walrus
    # rejects empty .instr with "ISA wrong length".
    mybir.codegen_inst_isa_subclasses(nc)
    return nc


def get_nc(reps=1, phase=None, ablate=None, dbg=False):
    key = ("nc", reps, phase, ablate, dbg)
    if key not in _CACHE:
        _CACHE[key] = _build_bass(reps, phase, ablate, dbg)
    return _CACHE[key]


def make_in_maps(inputs):
    ident = np.eye(128, dtype=np.float32)
    iotac = np.broadcast_to(
        (np.arange(NCHUNK, dtype=np.uint32) * SEL_CHUNK)
        .repeat(8)[None, :], (128, NCHUNK * 8)).copy()
    masku = np.broadcast_to(
        np.concatenate([np.full(64, 0x7FFFF000, np.uint32),
                        np.full(K, 0x00000FFF, np.uint32)])[None, :],
        (128, 64 + K)).copy()
    outs = np.ascontiguousarray(inputs["outputs"], np.float32)
    tgts = np.ascontiguousarray(inputs["targets"], np.float32)
    lat = np.ascontiguousarray(inputs["latent"], np.float32)
    rawf = np.ascontiguousarray(inputs["raw"], np.float32)
    maps = []
    for c in range(NCORES):
        sl = slice(c * RPC, (c + 1) * RPC)
        maps.append({
            "outputs_l": np.ascontiguousarray(outs[sl]),
            "targets_l": np.ascontiguousarray(tgts[sl]),
            "raw_l": np.ascontiguousarray(rawf[sl]),
            "latent": lat,
            "raw": rawf,
            "ident": ident,
            "iotac": iotac,
            "masku": masku,
        })
    return maps


def kernel(**inputs) -> np.ndarray:
    os.environ.setdefault("JAX_PLATFORMS", "")
    from concourse.bass_utils import run_bass_kernel_spmd

    nc = get_nc()
    in_maps = make_in_maps(inputs)
    r = run_bass_kernel_spmd(nc, in_maps, core_ids=list(range(NCORES)))
    total = np.float32(0.0)
    for dev in r.results:
        total = np.float32(total + np.float32(dev["res"].reshape(())))
    return np.asarray(total, dtype=np.float32)


if __name__ == "__main__":
    nc = get_nc()
    print("bass build OK:", nc)


# revision 50
# speedup vs baseline: 1.0079x; 1.0079x over previous
"""Trainium2 Bass kernel for nn_AllGeomLoss (retrieval_knn).

Self-contained: takes FULL inputs, shards rows across 8 NeuronCores
internally (data-parallel, 512 rows/core), returns the full scalar output.

Per-core algorithm (everything on-device except the final 8-way scalar sum):
  - recon: partial sum of (outputs-targets)^2 over this core's 512 rows.
  - global latent covariance C accumulated on PE (bf16 inputs) fused with
    the column-sum via a ones column; pr = 0.01*tr(C)^2/||C||_F^2 exactly;
    aniso via lambda_max from 5 matrix squarings + Rayleigh quotient.
  - kNN: biased negated-distance rows d'[i,j] = 2 raw_i.raw_j - |raw_j|^2
    (fp16 PSUM; +512 bias added later preserves ranking) via one PE matmul
    per 512-col chunk.  Per chunk max8 + max_index give top-8 values and
    indices; global candidate top-26 via index-embedding in the low 12
    mantissa bits of the 64 candidates, then 4 rounds of max8/
    match_replace8.  Rank 0 is self (d'[i,i] is the strict row max).
  - Neighbor gather: ONE batched gpsimd dma_gather per 128-row tile (3200
    int16 indices, 16-partition-wrapped and replicated), one SWDGE queue
    per tile so descriptor generation for the 4 tiles overlaps on the 8
    Q7 cores.  Index repack runs on PE (16-partition replicate matmuls) +
    one strided DVE cast-copy.
  - tsa: per-row top eigenvector of the 25-neighborhood covariance via one
    power iteration u = Yc^T(Yc v0), v0 = Y0 - Y1, computed for latent and
    raw sides JOINTLY on [128, 25, 128] tiles; tsa needs only
    (uz.ux)^2/(|uz|^2|ux|^2).
  - Emission order: all 4 tiles' selection+gather first, then cov/recon
    work (fills the gather window), then the 4 eig stages — so the
    in-order Vector queue never head-of-line blocks on a gather.
"""
import os
import numpy as np

B, D = 4096, 64
NCORES = 8
RPC = B // NCORES          # rows per core = 512
NT = RPC // 128            # 128-row tiles per core = 4
K = 25
SEL_CHUNK = 512            # selection chunk size
NCHUNK = B // SEL_CHUNK    # 8
KEPS = 1.0 / (B - 1 + 1e-8)

_CACHE = {}


def _apply_compiler_workarounds():
    # This container's walrus build rejects instructions carrying more than
    # one sync-wait (Drain at the kernel tail collects one wait per DMA
    # queue semaphore).  Collapse the HW/SW DGE round-robin to a single
    # semaphore lane and spread the tail-drain waits over one-wait nops.
    import concourse.tile_sem_assignment as _tsa
    import concourse.tile as _tile

    if not getattr(_tile.TileContext, "_drain_split_patched", False):
        _orig_dab = _tile.TileContext._drain_and_barrier

        def _drain_and_barrier_split(self, tick_clock, wait_clock):
            from concourse.vector_clock import ScopedClock, VectorClock
            gc = tick_clock.global_clock
            for p in range(_tsa.N_PROCS):
                if gc[p] > 0:
                    part = [0] * _tsa.N_PROCS
                    part[p] = gc[p]
                    nop = self.nc.sync.nop(nofuse=True)
                    wait_clock.add_sem_waits(
                        nop.ins, ScopedClock({None: VectorClock(part)}))
            self.nc.sync.drain()
            self.nc.all_engine_barrier()
            assert self.sems is not None
            popped = self.nc._tile_sem_poison_stack.pop()
            assert popped is self._sem_poison
            self.nc.clear_and_free_semaphores(
                list(self.sems.allocated().values()))
            self.nc.all_engine_barrier()

        _tile.TileContext._drain_and_barrier = _drain_and_barrier_split
        _tile.TileContext._drain_split_patched = True

    from concourse.bass import Bass as _Bass
    if not getattr(_Bass, "_json_wait_split_patched", False):
        _orig_to_json = _Bass.to_json_bytes

        def _to_json_split_waits(self, *a, **kw):
            import json as _json
            raw = _orig_to_json(self, *a, **kw)
            m = _json.loads(raw)
            changed = False
            for f in m.get("functions", []):
                for blk in f.get("blocks", []):
                    insts = blk.get("instructions")
                    if not insts:
                        continue
                    new = []
                    for ins in insts:
                        if ins.get("opcode") == "ISA" and \
                                ins.get("op_name") == "SeqAssert":
                            # This walrus build rejects SeqAssert encodings
                            # ("ISA wrong length"); our dynamic values are
                            # partition ids with statically-known range.
                            changed = True
                            ins = {
                                "debug": ins.get("debug", 0),
                                "engine": ins["engine"],
                                "ins": [],
                                "name": ins["name"],
                                "opcode": "NoOp",
                                "outs": [],
                                "sync_info": ins.get("sync_info") or
                                {"on_update": [], "on_wait": []},
                            }
                        si = ins.get("sync_info") or {}
                        ow = si.get("on_wait") or []
                        if len(ow) > 1:
                            changed = True
                            for j, w in enumerate(ow[:-1]):
                                new.append({
                                    "debug": ins.get("debug", 0),
                                    "engine": ins["engine"],
                                    "ins": [],
                                    "name": f"{ins['name']}_wsplit{j}",
                                    "opcode": "NoOp",
                                    "outs": [],
                                    "sync_info": {"on_update": [],
                                                  "on_wait": [w]},
                                })
                            si["on_wait"] = [ow[-1]]
                        new.append(ins)
                    blk["instructions"] = new
            if not changed:
                return raw
            return _json.dumps(m).encode()

        _Bass.to_json_bytes = _to_json_split_waits
        _Bass._json_wait_split_patched = True


def _build_bass(reps=1, phase=None, ablate=None, dbg=False):
    from concourse.bass import Bass
    from concourse import mybir
    from concourse import library_config
    from concourse.tile import TileContext
    from contextlib import ExitStack

    _apply_compiler_workarounds()

    f32 = mybir.dt.float32
    f16 = mybir.dt.float16
    bf16 = mybir.dt.bfloat16
    u32 = mybir.dt.uint32
    i32 = mybir.dt.int32
    i16 = mybir.dt.int16

    nc = Bass(trn_type="TRN2", enable_asserts=False, num_swdge_queues=4)

    outputs_l = nc.dram_tensor("outputs_l", [RPC, D], f32, kind="ExternalInput")
    targets_l = nc.dram_tensor("targets_l", [RPC, D], f32, kind="ExternalInput")
    raw_l = nc.dram_tensor("raw_l", [RPC, D], f32, kind="ExternalInput")
    latent = nc.dram_tensor("latent", [B, D], f32, kind="ExternalInput")
    raw = nc.dram_tensor("raw", [B, D], f32, kind="ExternalInput")
    ident_in = nc.dram_tensor("ident", [128, 128], f32, kind="ExternalInput")
    iota_in = nc.dram_tensor("iotac", [128, NCHUNK * 8], u32,
                             kind="ExternalInput")
    # bitmask constants as tensor_tensor operands: cols 0:64 = 0x7FFFF000
    # (embed mask), cols 64:89 = 0xFFF (index decode mask).  DVE
    # tensor_scalar can enter 2-port perf mode and then fully blocks
    # against active SWDGE descriptor generation (the gathers);
    # tensor_tensor never contends, so masks come in as tensors.
    mask_in = nc.dram_tensor("masku", [128, 64 + K], u32,
                             kind="ExternalInput")
    res = nc.dram_tensor("res", [1, 1], f32, kind="ExternalOutput")
    combD = nc.dram_tensor("combD", [B, 2 * D], bf16)
    if dbg:
        dbg_ch = nc.dram_tensor("dbg_ch", [128, SEL_CHUNK], f32,
                                kind="ExternalOutput")
        dbg_cv = nc.dram_tensor("dbg_cv", [128, NCHUNK * 8], f32,
                                kind="ExternalOutput")
        dbg_ci = nc.dram_tensor("dbg_ci", [128, NCHUNK * 8], i32,
                                kind="ExternalOutput")
        dbg_idx = nc.dram_tensor("dbg_idx", [128, K], i32,
                                 kind="ExternalOutput")
        dbg_comb = nc.dram_tensor("dbg_comb", [128, K * 2 * D], f32,
                                  kind="ExternalOutput")
        dbg_stats = nc.dram_tensor("dbg_stats", [128, 8], f32,
                                   kind="ExternalOutput")
        dbg_u = nc.dram_tensor("dbg_u", [128, 2 * D], f32,
                               kind="ExternalOutput")
        dbg_sv = nc.dram_tensor("dbg_sv", [128, K * 2], f32,
                                kind="ExternalOutput")

    A = mybir.AluOpType
    AX = mybir.AxisListType

    def pr(name):
        return reps if phase == name else 1

    with nc.allow_low_precision("bf16/fp16 distance/eig stages within tol"), \
            TileContext(nc) as tc, ExitStack() as ctx:

        const_p = ctx.enter_context(tc.tile_pool(name="const", bufs=1))
        chunk_p = ctx.enter_context(tc.tile_pool(name="chunk", bufs=3))
        sel_p = ctx.enter_context(tc.tile_pool(name="sel", bufs=4))
        eig_p = ctx.enter_context(tc.tile_pool(name="eig", bufs=2))
        cg_p = ctx.enter_context(tc.tile_pool(name="cgp", bufs=4))
        psum_p = ctx.enter_context(tc.tile_pool(name="psum", bufs=4, space="PSUM"))
        psS = ctx.enter_context(tc.tile_pool(name="psS", bufs=2, space="PSUM"))
        cov_p = ctx.enter_context(tc.tile_pool(name="covp", bufs=1, space="PSUM"))

        # ---- constants ----
        ident = const_p.tile([128, 128], f32)
        identb = const_p.tile([128, 128], bf16)
        stats = const_p.tile([128, 8], f32)
        ones64b = const_p.tile([64, 1], bf16)
        ones64f = const_p.tile([64, 1], f32)
        ones128 = const_p.tile([128, 1], f32)
        iota_off = const_p.tile([128, NCHUNK * 8], u32)
        masks = const_p.tile([128, 64 + K], u32)
        negK = const_p.tile([128, 1], f32)
        kepsc = const_p.tile([64, 1], f32)

        # dma_gather (InstDMAGatherAnt) lives in the 'mlp' gpsimd library;
        # iota was replaced with a host-supplied constant so no standard-
        # library op remains and one load at kernel start suffices.
        nc.gpsimd.load_library(library_config.mlp)
        nc.sync.dma_start(ident[:], ident_in[:])
        nc.sync.dma_start(iota_off[:], iota_in[:])
        nc.sync.dma_start(masks[:], mask_in[:])
        nc.vector.tensor_copy(identb[:], ident[:])
        nc.vector.memset(ones64b[:], 1.0)
        nc.vector.memset(ones64f[:], 1.0)
        nc.vector.memset(ones128[:], 1.0)
        nc.vector.memset(stats[:], 0.0)
        nc.vector.memset(negK[:], -1.0 / K)
        nc.vector.memset(kepsc[:], KEPS)
        # Rt[:, g, :] is the [128, 128] f32 stationary R_g with
        # R_g[c, q] = 1 iff c == g*16 + q%16.  matmul(lhsT=R_g, rhs=idxf)
        # replicates rows [16g, 16g+16) of idxf onto all 8 16-partition
        # groups — the dma_gather index buffer wants the 16-partition wrap
        # replicated into every Q7 core window (queue q reads partitions
        # [32q, 32q+32), so full replication covers all queues).
        Rt = const_p.tile([128, 8, 128], f32)
        for g in range(8):
            nc.vector.tensor_copy(
                Rt[:, g, :].rearrange("p (k q) -> p k q", k=8),
                ident[:, 16 * g:16 * (g + 1)].unsqueeze(1)
                .broadcast_to([128, 8, 16]))

        for _rep in range(reps if phase is None else 1):
            # ---- prep: two full-table loads (p-major: 8KB descriptors),
            # bf16 comb table [1 | latent | raw], X matrix ----
            # global row j = 32*p + t  (partition-major layout)
            raw_f = const_p.tile([128, 32, D], f32, tag="rawf")
            lat_f = const_p.tile([128, 32, D], f32, tag="latf")
            comb = const_p.tile([128, 32, 2 * D + 1], bf16, tag="comb")
            nc.sync.dma_start(raw_f[:],
                              raw[:].rearrange("(p t) d -> p t d", p=128))
            nc.sync.dma_start(lat_f[:],
                              latent[:].rearrange("(p t) d -> p t d", p=128))
            # recon + local-raw loads issued early so the DMA overlaps prep
            ob = const_p.tile([128, NT, 64], f32, tag="ob")
            tb = const_p.tile([128, NT, 64], f32, tag="tb")
            nc.sync.dma_start(ob[:],
                              outputs_l[:].rearrange("(p t) d -> p t d", t=NT))
            nc.sync.dma_start(tb[:],
                              targets_l[:].rearrange("(p t) d -> p t d", t=NT))
            rloc = const_p.tile([128, NT, 64], f32, tag="rloc")
            nc.sync.dma_start(rloc[:],
                              raw_l[:].rearrange("(p t) d -> p t d", t=NT))

            nc.vector.memset(comb[:, :, 0:1], 1.0)
            nc.vector.tensor_copy(comb[:, :, 1:D + 1], lat_f[:])
            nc.vector.tensor_copy(comb[:, :, D + 1:2 * D + 1], raw_f[:])
            # combined bf16 table to DRAM (for the gathers); row j = 32p+t
            nc.sync.dma_start(
                combD[:].rearrange("(p t) c -> p t c", p=128, t=32),
                comb[:, :, 1:2 * D + 1])
            rlocb = const_p.tile([128, NT, 64], bf16, tag="rlocb")
            nc.vector.tensor_copy(rlocb[:], rloc[:])

            # ---- X = [rawT (64 rows); 512-sq] [65, 4096] bf16 ----
            # the +512 embed bias is folded into the bias row (ACT bias on
            # the sq write): the DVE tensor_scalar add it replaces blocks
            # against active SWDGE descriptor generation.
            # transpose of comb tile t gives raw rows {32p+t} -> X cols 32p+t
            X = const_p.tile([65, B], bf16)
            # Wb stationaries [65, 128]: rows 0:64 = 2*rawT_local (per
            # tile), row 64 = 1 (pairs 512-sq).  The constant row is
            # memset here, before any gather runs.
            Wbs = []
            for t in range(NT):
                Wb = sel_p.tile([65, 128], bf16, tag=f"Wb{t}")
                nc.vector.memset(Wb[64:65, :], 1.0)
                Wbs.append(Wb)
            Xv = X[0:64, :].rearrange("q (p t) -> q t p", t=32)
            for g in range(8):
                pT_ps = psS.tile([64, 4, 128], bf16, tag="s", space="PSUM")
                for u in range(4):
                    nc.tensor.transpose(out=pT_ps[:, u, :],
                                        in_=comb[:, 4 * g + u, D + 1:2 * D + 1],
                                        identity=identb[:])
                nc.scalar.copy(Xv[:, 4 * g:4 * g + 4, :], pT_ps[:])
            for c in range(NCHUNK):
                cs = slice(c * SEL_CHUNK, (c + 1) * SEL_CHUNK)
                sq_t = chunk_p.tile([64, SEL_CHUNK], bf16, tag="sqt")
                nc.vector.tensor_mul(sq_t[:], X[0:64, cs], X[0:64, cs])
                sq_ps = psS.tile([1, SEL_CHUNK], f32, tag="s", space="PSUM")
                nc.tensor.matmul(out=sq_ps[:], lhsT=ones64b[:], rhs=sq_t[:],
                                 start=True, stop=True)
                # X row 64 = 512 - |r_j|^2 (bias bakes the embed offset in)
                nc.scalar.activation(X[64:65, cs], sq_ps[:],
                                     mybir.ActivationFunctionType.Copy,
                                     bias=512.0, scale=-1.0)

            # ---- per 128-row tile: selection + batched gather ----
            def emit_selgather(t):
                Wb = Wbs[t]
                rT2_ps = psS.tile([64, 128], bf16, tag="s", space="PSUM")
                nc.tensor.transpose(out=rT2_ps[:], in_=rlocb[:, t, :],
                                    identity=identb[:])
                nc.scalar.mul(Wb[0:64, :], rT2_ps[:], 2.0)

                cand_v = sel_p.tile([128, NCHUNK * 8], f32, tag="cand_v")
                cand_i = sel_p.tile([128, NCHUNK * 8], u32, tag="cand_i")
                for c in range(NCHUNK):
                    ps_d = psum_p.tile([128, SEL_CHUNK], f32, tag="dist",
                                       space="PSUM")
                    for _dr in range(pr("dist")):
                        nc.tensor.matmul(
                            out=ps_d[:], lhsT=Wb[:],
                            rhs=X[:, c * SEL_CHUNK:(c + 1) * SEL_CHUNK],
                            start=True, stop=True)
                    if dbg and t == 0 and c == 0:
                        chf = chunk_p.tile([128, SEL_CHUNK], f32, tag="chf")
                        nc.vector.tensor_copy(chf[:], ps_d[:])
                        nc.sync.dma_start(dbg_ch[:], chf[:])
                    for _sr in range(pr("sel")):
                        nc.vector.max(cand_v[:, c * 8:(c + 1) * 8], ps_d[:])
                        nc.vector.max_index(cand_i[:, c * 8:(c + 1) * 8],
                                            cand_v[:, c * 8:(c + 1) * 8],
                                            ps_d[:])
                # embed global index into low 12 mantissa bits of the 64
                # candidates (+512 bias first: values ~[350,620),
                # quantum <= 0.25)
                emb = sel_p.tile([128, NCHUNK * 8], u32, tag="emb")
                top32 = sel_p.tile([128, 32], u32, tag="top32")
                idx32 = sel_p.tile([128, K], i32, tag="idx32")
                for _sr in range(pr("sel")):
                    nc.vector.tensor_tensor(out=cand_i[:], in0=cand_i[:],
                                            in1=iota_off[:], op=A.add)
                    # cand_v already carries the +512 bias from the matmul
                    nc.vector.tensor_tensor(out=emb[:],
                                            in0=cand_v[:].bitcast(u32),
                                            in1=masks[:, 0:64],
                                            op=A.bitwise_and)
                    nc.vector.tensor_tensor(out=emb[:], in0=emb[:],
                                            in1=cand_i[:], op=A.bitwise_or)
                    for r in range(4):
                        nc.vector.max(top32[:, r * 8:(r + 1) * 8].bitcast(f32),
                                      emb[:].bitcast(f32))
                        if r < 3:
                            nc.vector.match_replace(
                                out=emb[:].bitcast(f32),
                                in_to_replace=top32[:, r * 8:(r + 1) * 8]
                                .bitcast(f32),
                                in_values=emb[:].bitcast(f32), imm_value=0.0)
                    # decode 25 neighbor indices (drop rank 0 = self)
                    nc.vector.tensor_tensor(out=idx32[:].bitcast(u32),
                                            in0=top32[:, 1:1 + K],
                                            in1=masks[:, 64:64 + K],
                                            op=A.bitwise_and)

                if dbg and t == 0:
                    nc.sync.dma_start(dbg_cv[:], cand_v[:])
                    nc.sync.dma_start(dbg_ci[:], cand_i[:].bitcast(i32))
                    nc.sync.dma_start(dbg_idx[:], idx32[:])
                comb_g = cg_p.tile([128, K, 2 * D], bf16, tag="comb_g")
                if ablate == "gather":
                    nc.vector.memset(comb_g[:], 1.0)
                else:
                    # ONE batched dma_gather per tile.  dma_gather reads
                    # int16 indices wrapped in 16 partitions (linear
                    # n = s*16 + p), replicated to every 16-partition
                    # group, and writes gathered row n to
                    # dst[n % 128, n // 128, :]: with n = a*128 + i the
                    # index at [p, 8a + g] must be idx32[g*16 + p, a].
                    # E16 matmuls replicate rows 16g..16g+16 across all
                    # partition groups; the strided cast-copy interleaves
                    # (g, a) -> column a*8 + g.
                    # both casts run on the Scalar engine so the gather's
                    # input chain never sits behind eig work in the
                    # in-order Vector queue.
                    idxf = sel_p.tile([128, K], f32, tag="idxf")
                    nc.scalar.copy(idxf[:], idx32[:])
                    rep_ps = psS.tile([128, 8, K], f32, tag="s", space="PSUM")
                    for g in range(8):
                        nc.tensor.matmul(out=rep_ps[:, g, :],
                                         lhsT=Rt[:, g, :], rhs=idxf[:],
                                         start=True, stop=True)
                    idxs16 = sel_p.tile([128, 8 * K], i16, tag="idxs16")
                    nc.scalar.copy(
                        idxs16[:].rearrange("p (a g) -> p g a", g=8),
                        rep_ps[:])
                    for _gr in range(pr("gather")):
                        # single_packet=False: coalescing 3200 descs into
                        # one packet per engine exceeds the <=64-descriptor
                        # packet ceiling and wedges the device.  One SWDGE
                        # queue per tile: descriptor generation runs on a
                        # different Q7 core pair per queue and overlaps.
                        nc.gpsimd.dma_gather(
                            out_ap=comb_g[:], in_ap=combD[:],
                            idxs_ap=idxs16[:], num_idxs=128 * K,
                            num_idxs_reg=128 * K, elem_size=2 * D,
                            single_packet=False, queue_num=t)

                if dbg and t == 0:
                    cgf = eig_p.tile([128, K * 2 * D], f32, tag="cgf")
                    nc.vector.tensor_copy(
                        cgf[:], comb_g[:].rearrange("p k c -> p (k c)"))
                    nc.sync.dma_start(dbg_comb[:], cgf[:])
                return comb_g, idx32

            # ---- eig: both sides jointly; one power iteration ----
            eig_count = [0]

            def emit_eig(comb_g, gate_b):
                first_eig = eig_count[0] == 0
                eig_count[0] += 1
                for _er in range(pr("eig")):
                    # gate_b is all-zeros, produced after the LAST tile's
                    # index decode: a real dependency on the FIRST op of
                    # the eig chain keeps every eig Vector op behind all
                    # selection/decode Vector ops in any schedule (the
                    # in-order Vector queue would otherwise head-of-line
                    # block later selections — and thus the gathers — on
                    # this eig's gather data).  All other eig ops depend
                    # on v0 or its descendants.
                    v0 = eig_p.tile([128, 2 * D], bf16, tag="v0")
                    nc.vector.tensor_tensor(
                        out=v0[:], in0=comb_g[:, 0, :],
                        in1=gate_b[:].broadcast_to([128, 2 * D]), op=A.add)
                    nc.vector.tensor_sub(v0[:], v0[:], comb_g[:, 1, :])
                    t1 = eig_p.tile([128, K, 2 * D], bf16, tag="t1")
                    nc.vector.tensor_tensor(
                        out=t1[:], in0=comb_g[:],
                        in1=v0[:].unsqueeze(1).broadcast_to([128, K, 2 * D]),
                        op=A.mult)
                    # per-(neighbor, side) dots: reduce innermost 64
                    s_v = eig_p.tile([128, K, 2], f32, tag="sv")
                    nc.vector.tensor_reduce(
                        out=s_v[:],
                        in_=t1[:].rearrange("p k (s d) -> p k s d", s=2),
                        axis=AX.X, op=A.add)
                    ssum = eig_p.tile([128, 2], f32, tag="ssum")
                    nc.vector.tensor_reduce(
                        out=ssum[:], in_=s_v[:].rearrange("p k s -> p s k"),
                        axis=AX.X, op=A.add)
                    # center: s = s - mean_k(s), via two tensor_tensor ops
                    # (scalar_tensor_tensor may enter the 2-port perf mode
                    # that blocks against active SWDGE generation)
                    nc.vector.tensor_tensor(
                        out=ssum[:], in0=ssum[:],
                        in1=negK[:].broadcast_to([128, 2]), op=A.mult)
                    nc.vector.tensor_tensor(
                        out=s_v[:],
                        in0=ssum[:].unsqueeze(1).broadcast_to([128, K, 2]),
                        in1=s_v[:], op=A.add)
                    # t2 = Y * s_bc, then tree-reduce over k
                    t2 = eig_p.tile([128, K + 7, 2 * D], bf16, tag="t2")
                    nc.vector.tensor_tensor(
                        out=t2[:, 0:K, :].rearrange("p k (s d) -> p k s d", s=2),
                        in0=comb_g[:].rearrange("p k (s d) -> p k s d", s=2),
                        in1=s_v[:].unsqueeze(3).broadcast_to([128, K, 2, D]),
                        op=A.mult)
                    n = K
                    while n > 1:
                        h = n // 2
                        nc.vector.tensor_add(t2[:, 0:h, :], t2[:, 0:h, :],
                                             t2[:, h:2 * h, :])
                        if n % 2:
                            # move via add-zero: tensor_copy can enter the
                            # 2-port mode that blocks against SWDGE
                            nc.vector.tensor_tensor(
                                out=t2[:, h:h + 1, :],
                                in0=t2[:, n - 1:n, :],
                                in1=gate_b[:].unsqueeze(1)
                                .broadcast_to([128, 1, 2 * D]), op=A.add)
                            n = h + 1
                        else:
                            n = h
                    if dbg and first_eig:
                        svf = eig_p.tile([128, K * 2], f32, tag="svf")
                        nc.vector.tensor_copy(
                            svf[:], s_v[:].rearrange("p k s -> p (k s)"))
                        nc.sync.dma_start(dbg_sv[:], svf[:])
                    # overlap stats: q = (uz.ux)^2 / (|uz|^2 |ux|^2)
                    u = t2[:, 0, :]
                    u2 = eig_p.tile([128, 2 * D], f32, tag="u2")
                    nc.vector.tensor_mul(u2[:], u, u)
                    nn_v = eig_p.tile([128, 2], f32, tag="nn")
                    nc.vector.tensor_reduce(
                        out=nn_v[:], in_=u2[:].rearrange("p (s d) -> p s d", s=2),
                        axis=AX.X, op=A.add)
                    cr = eig_p.tile([128, D], f32, tag="cr")
                    nc.vector.tensor_mul(cr[:], u[:, 0:D], u[:, D:2 * D])
                    dzx = eig_p.tile([128, 2], f32, tag="dzx")
                    nc.vector.tensor_reduce(out=dzx[:, 0:1], in_=cr[:],
                                            axis=AX.X, op=A.add)
                    nc.vector.tensor_mul(dzx[:, 1:2], nn_v[:, 0:1], nn_v[:, 1:2])
                    # +1 guards div-by-zero for degenerate rows (duplicate
                    # neighbors from exact distance ties); den is ~1e10
                    # normally so the bias is negligible.
                    nc.vector.tensor_add(dzx[:, 1:2], dzx[:, 1:2],
                                         ones128[:, 0:1])
                    nc.vector.reciprocal(dzx[:, 1:2], dzx[:, 1:2])
                    nc.vector.tensor_mul(dzx[:, 0:1], dzx[:, 0:1], dzx[:, 0:1])
                    nc.vector.tensor_mul(dzx[:, 0:1], dzx[:, 0:1], dzx[:, 1:2])
                    nc.vector.tensor_add(stats[:, 1:2], stats[:, 1:2],
                                         dzx[:, 0:1])
                    if dbg and first_eig:
                        uf = eig_p.tile([128, 2 * D], f32, tag="uf")
                        nc.vector.tensor_copy(uf[:], u)
                        nc.sync.dma_start(dbg_u[:], uf[:])

            # all 4 selections+gathers first: the gathers start as early as
            # possible and run on their own SWDGE queues while the Vector
            # engine keeps busy with the following selections + cov/recon.
            sel_out = [emit_selgather(t) for t in range(NT)]
            cgs = [cg for cg, _ in sel_out]
            # zero gate derived from the LAST tile's decoded indices
            # (x - x == 0; tensor_tensor form never contends with SWDGE)
            gate_b = const_p.tile([128, 1], bf16, tag="gate")
            nc.vector.tensor_tensor(out=gate_b[:], in0=sel_out[-1][1][:, 0:1],
                                    in1=sel_out[-1][1][:, 0:1],
                                    op=A.subtract)

            # ---- global latent covariance on PE (bf16), fused with the
            # column-sum via the ones column: out [64, 65] ----
            cov_ps = cov_p.tile([64, 65], f32, space="PSUM")
            for t in range(32):
                nc.tensor.matmul(out=cov_ps[:], lhsT=comb[:, t, 1:D + 1],
                                 rhs=comb[:, t, 0:D + 1],
                                 start=(t == 0), stop=(t == 31))

            # ---- cov postprocessing: C, trC, trC2, lambda_max ----
            cov_s = const_p.tile([64, 65], f32, tag="covs")
            nc.scalar.copy(cov_s[:], cov_ps[:])
            # s as a row: s_row[0, f] = s[f] via lhsT = s_col
            srow_ps = psS.tile([1, 64], f32, tag="s", space="PSUM")
            nc.tensor.matmul(out=srow_ps[:], lhsT=cov_s[:, 0:1],
                             rhs=ident[0:64, 0:64], start=True, stop=True)
            s_row = const_p.tile([1, 64], f32, tag="srow")
            nc.scalar.copy(s_row[:], srow_ps[:])
            ssT_ps = psS.tile([64, 64], f32, tag="s", space="PSUM")
            nc.tensor.matmul(out=ssT_ps[:], lhsT=s_row[:], rhs=s_row[:],
                             start=True, stop=True)
            sst_s = const_p.tile([64, 64], f32, tag="sst")
            nc.scalar.mul(sst_s[:], ssT_ps[:], KEPS / B)
            C_s = const_p.tile([64, 64], f32, tag="Cs")
            # C = cov*KEPS - ssT*(KEPS/B), via tensor_tensor forms only
            # (this block overlaps the gathers)
            nc.vector.tensor_tensor(out=C_s[:], in0=cov_s[:, 1:D + 1],
                                    in1=kepsc[:].broadcast_to([64, 64]),
                                    op=A.mult)
            nc.vector.tensor_sub(C_s[:], C_s[:], sst_s[:])
            diag_scr = const_p.tile([64, 64], f32, tag="dscr")
            nc.vector.tensor_mul(diag_scr[:], C_s[:], ident[0:64, 0:64])
            nc.vector.tensor_reduce(out=stats[0:64, 2:3], in_=diag_scr[:],
                                    axis=AX.X, op=A.add)
            fro_scr = const_p.tile([64, 64], f32, tag="fscr")
            nc.vector.tensor_mul(fro_scr[:], C_s[:], C_s[:])
            nc.vector.tensor_reduce(out=stats[0:64, 3:4], in_=fro_scr[:],
                                    axis=AX.X, op=A.add)
            # 5 squarings: M = C^32, then Rayleigh via w = M.1
            M_prev = C_s
            for sqi in range(5):
                m_ps = psS.tile([64, 64], f32, tag="s", space="PSUM")
                nc.tensor.matmul(out=m_ps[:], lhsT=M_prev[:], rhs=M_prev[:],
                                 start=True, stop=True)
                M_new = const_p.tile([64, 64], f32, tag=f"m{sqi}")
                nc.scalar.copy(M_new[:], m_ps[:])
                M_prev = M_new
            w_ps = psS.tile([64, 1], f32, tag="s", space="PSUM")
            nc.tensor.matmul(out=w_ps[:], lhsT=M_prev[:], rhs=ones64f[:],
                             start=True, stop=True)
            w_s = const_p.tile([64, 1], f32, tag="ws")
            nc.scalar.copy(w_s[:], w_ps[:])
            r_ps = psS.tile([64, 1], f32, tag="s", space="PSUM")
            nc.tensor.matmul(out=r_ps[:], lhsT=C_s[:], rhs=w_s[:],
                             start=True, stop=True)
            nc.vector.tensor_mul(stats[0:64, 4:5], w_s[:], r_ps[:])
            nc.vector.tensor_mul(stats[0:64, 5:6], w_s[:], w_s[:])

            # ---- recon over this core's 512-row slice ----
            dif = const_p.tile([128, NT, 64], f32, tag="dif")
            nc.vector.tensor_sub(dif[:], ob[:], tb[:])
            nc.vector.tensor_mul(dif[:], dif[:], dif[:])
            nc.vector.tensor_reduce(out=stats[:, 0:1], in_=dif[:],
                                    axis=AX.XY, op=A.add)

            # ---- eig stages, in gather-completion order ----
            if ablate != "eig":
                for t in range(NT):
                    emit_eig(cgs[t], gate_b)

        if dbg:
            nc.sync.dma_start(dbg_stats[:], stats[:])
        # ---- final scalar assembly ----
        fin_ps = psS.tile([1, 8], f32, tag="s", space="PSUM")
        nc.tensor.matmul(out=fin_ps[:], lhsT=ones128[:], rhs=stats[:],
                         start=True, stop=True)
        fin = const_p.tile([1, 8], f32, tag="fin")
        nc.scalar.copy(fin[:], fin_ps[:])
        sc = const_p.tile([1, 8], f32, tag="sc")
        res_s = const_p.tile([1, 1], f32, tag="ress")
        nc.vector.reciprocal(sc[:, 0:1], fin[:, 3:4])          # 1/trC2
        nc.vector.reciprocal(sc[:, 1:2], fin[:, 5:6])          # 1/(w.w)
        nc.vector.reciprocal(sc[:, 2:3], fin[:, 2:3])          # 1/trC
        nc.vector.tensor_mul(sc[:, 3:4], fin[:, 2:3], fin[:, 2:3])
        nc.vector.tensor_mul(sc[:, 3:4], sc[:, 3:4], sc[:, 0:1])   # pr ratio
        nc.vector.tensor_mul(sc[:, 4:5], fin[:, 4:5], sc[:, 1:2])  # lambda
        nc.vector.tensor_mul(sc[:, 4:5], sc[:, 4:5], sc[:, 2:3])   # lam/trC
        # S = f0/262144 + 0.02625 - (0.2/4096) f1 + 0.00125 pr - 0.00125 q
        nc.vector.tensor_scalar(res_s[:], fin[:, 0:1], 1.0 / (B * D), 0.02625,
                                op0=A.mult, op1=A.add)
        nc.vector.scalar_tensor_tensor(out=res_s[:], in0=fin[:, 1:2],
                                       scalar=-0.2 / B, in1=res_s[:],
                                       op0=A.mult, op1=A.add)
        nc.vector.scalar_tensor_tensor(out=res_s[:], in0=sc[:, 3:4],
                                       scalar=0.00125, in1=res_s[:],
                                       op0=A.mult, op1=A.add)
        nc.vector.scalar_tensor_tensor(out=res_s[:], in0=sc[:, 4:5],
                                       scalar=-0.00125, in1=res_s[:],
                                       op0=A.mult, op1=A.add)
        nc.sync.dma_start(res[:], res_s[:])

    # Raw Bass skips Bacc.compile(); fill in the ISA encoding bytes for
    # extended-inst ISA subclasses (PseudoReloadLibraryIndex) — walrus
    # rejects empty .instr with "ISA wrong length".
    mybir.codegen_inst_isa_subclasses(nc)
    return nc


def get_nc(reps=1, phase=None, ablate=None, dbg=False):
    key = ("nc", reps, phase, ablate, dbg)
    if key not in _CACHE:
        _CACHE[key] = _build_bass(reps, phase, ablate, dbg)
    return _CACHE[key]


def make_in_maps(inputs):
    ident = np.eye(128, dtype=np.float32)
    iotac = np.broadcast_to(
        (np.arange(NCHUNK, dtype=np.uint32) * SEL_CHUNK)
        .repeat(8)[None, :], (128, NCHUNK * 8)).copy()
    masku = np.broadcast_to(
        np.concatenate([np.full(64, 0x7FFFF000, np.uint32),
                        np.full(K, 0x00000FFF, np.uint32)])[None, :],
        (128, 64 + K)).copy()
    outs = np.ascontiguousarray(inputs["outputs"], np.float32)
    tgts = np.ascontiguousarray(inputs["targets"], np.float32)
    lat = np.ascontiguousarray(inputs["latent"], np.float32)
    rawf = np.ascontiguousarray(inputs["raw"], np.float32)
    maps = []
    for c in range(NCORES):
        sl = slice(c * RPC, (c + 1) * RPC)
        maps.append({
            "outputs_l": np.ascontiguousarray(outs[sl]),
            "targets_l": np.ascontiguousarray(tgts[sl]),
            "raw_l": np.ascontiguousarray(rawf[sl]),
            "latent": lat,
            "raw": rawf,
            "ident": ident,
            "iotac": iotac,
            "masku": masku,
        })
    return maps


def kernel(**inputs) -> np.ndarray:
    os.environ.setdefault("JAX_PLATFORMS", "")
    from concourse.bass_utils import run_bass_kernel_spmd

    nc = get_nc()
    in_maps = make_in_maps(inputs)
    r = run_bass_kernel_spmd(nc, in_maps, core_ids=list(range(NCORES)))
    total = np.float32(0.0)
    for dev in r.results:
        total = np.float32(total + np.float32(dev["res"].reshape(())))
    return np.asarray(total, dtype=np.float32)


if __name__ == "__main__":
    nc = get_nc()
    print("bass build OK:", nc)


# revision 56
# speedup vs baseline: 1.0174x; 1.0094x over previous
"""Trainium2 Bass kernel for nn_AllGeomLoss (retrieval_knn).

Self-contained: takes FULL inputs, shards rows across 8 NeuronCores
internally (data-parallel, 512 rows/core), returns the full scalar output.

Per-core algorithm (everything on-device except the final 8-way scalar sum):
  - recon: partial sum of (outputs-targets)^2 over this core's 512 rows.
  - global latent covariance C accumulated on PE (bf16 inputs) fused with
    the column-sum via a ones column; pr = 0.01*tr(C)^2/||C||_F^2 exactly;
    aniso via lambda_max from 5 matrix squarings + Rayleigh quotient.
  - kNN: biased negated-distance rows d'[i,j] = 2 raw_i.raw_j - |raw_j|^2
    (fp16 PSUM; +512 bias added later preserves ranking) via one PE matmul
    per 512-col chunk.  Per chunk max8 + max_index give top-8 values and
    indices; global candidate top-26 via index-embedding in the low 12
    mantissa bits of the 64 candidates, then 4 rounds of max8/
    match_replace8.  Rank 0 is self (d'[i,i] is the strict row max).
  - Neighbor gather: ONE batched gpsimd dma_gather per 128-row tile (3200
    int16 indices, 16-partition-wrapped and replicated), one SWDGE queue
    per tile so descriptor generation for the 4 tiles overlaps on the 8
    Q7 cores.  Index repack runs on PE (16-partition replicate matmuls) +
    one strided DVE cast-copy.
  - tsa: per-row top eigenvector of the 25-neighborhood covariance via one
    power iteration u = Yc^T(Yc v0), v0 = Y0 - Y1, computed for latent and
    raw sides JOINTLY on [128, 25, 128] tiles; tsa needs only
    (uz.ux)^2/(|uz|^2|ux|^2).
  - Emission order: all 4 tiles' selection+gather first, then cov/recon
    work (fills the gather window), then the 4 eig stages — so the
    in-order Vector queue never head-of-line blocks on a gather.
"""
import os
import numpy as np

B, D = 4096, 64
NCORES = 8
RPC = B // NCORES          # rows per core = 512
NT = RPC // 128            # 128-row tiles per core = 4
K = 25
SEL_CHUNK = 512            # selection chunk size
NCHUNK = B // SEL_CHUNK    # 8
KEPS = 1.0 / (B - 1 + 1e-8)

_CACHE = {}


def _apply_compiler_workarounds():
    # This container's walrus build rejects instructions carrying more than
    # one sync-wait (Drain at the kernel tail collects one wait per DMA
    # queue semaphore).  Collapse the HW/SW DGE round-robin to a single
    # semaphore lane and spread the tail-drain waits over one-wait nops.
    import concourse.tile_sem_assignment as _tsa
    import concourse.tile as _tile

    if not getattr(_tile.TileContext, "_drain_split_patched", False):
        _orig_dab = _tile.TileContext._drain_and_barrier

        def _drain_and_barrier_split(self, tick_clock, wait_clock):
            from concourse.vector_clock import ScopedClock, VectorClock
            gc = tick_clock.global_clock
            for p in range(_tsa.N_PROCS):
                if gc[p] > 0:
                    part = [0] * _tsa.N_PROCS
                    part[p] = gc[p]
                    nop = self.nc.sync.nop(nofuse=True)
                    wait_clock.add_sem_waits(
                        nop.ins, ScopedClock({None: VectorClock(part)}))
            self.nc.sync.drain()
            self.nc.all_engine_barrier()
            assert self.sems is not None
            popped = self.nc._tile_sem_poison_stack.pop()
            assert popped is self._sem_poison
            self.nc.clear_and_free_semaphores(
                list(self.sems.allocated().values()))
            self.nc.all_engine_barrier()

        _tile.TileContext._drain_and_barrier = _drain_and_barrier_split
        _tile.TileContext._drain_split_patched = True

    from concourse.bass import Bass as _Bass
    if not getattr(_Bass, "_json_wait_split_patched", False):
        _orig_to_json = _Bass.to_json_bytes

        def _to_json_split_waits(self, *a, **kw):
            import json as _json
            raw = _orig_to_json(self, *a, **kw)
            m = _json.loads(raw)
            changed = False
            for f in m.get("functions", []):
                for blk in f.get("blocks", []):
                    insts = blk.get("instructions")
                    if not insts:
                        continue
                    new = []
                    for ins in insts:
                        if ins.get("opcode") == "ISA" and \
                                ins.get("op_name") == "SeqAssert":
                            # This walrus build rejects SeqAssert encodings
                            # ("ISA wrong length"); our dynamic values are
                            # partition ids with statically-known range.
                            changed = True
                            ins = {
                                "debug": ins.get("debug", 0),
                                "engine": ins["engine"],
                                "ins": [],
                                "name": ins["name"],
                                "opcode": "NoOp",
                                "outs": [],
                                "sync_info": ins.get("sync_info") or
                                {"on_update": [], "on_wait": []},
                            }
                        si = ins.get("sync_info") or {}
                        ow = si.get("on_wait") or []
                        if len(ow) > 1:
                            changed = True
                            for j, w in enumerate(ow[:-1]):
                                new.append({
                                    "debug": ins.get("debug", 0),
                                    "engine": ins["engine"],
                                    "ins": [],
                                    "name": f"{ins['name']}_wsplit{j}",
                                    "opcode": "NoOp",
                                    "outs": [],
                                    "sync_info": {"on_update": [],
                                                  "on_wait": [w]},
                                })
                            si["on_wait"] = [ow[-1]]
                        new.append(ins)
                    blk["instructions"] = new
            if not changed:
                return raw
            return _json.dumps(m).encode()

        _Bass.to_json_bytes = _to_json_split_waits
        _Bass._json_wait_split_patched = True


def _build_bass(reps=1, phase=None, ablate=None, dbg=False):
    from concourse.bass import Bass
    from concourse import mybir
    from concourse import library_config
    from concourse.tile import TileContext
    from contextlib import ExitStack

    _apply_compiler_workarounds()

    f32 = mybir.dt.float32
    f16 = mybir.dt.float16
    bf16 = mybir.dt.bfloat16
    u32 = mybir.dt.uint32
    i32 = mybir.dt.int32
    i16 = mybir.dt.int16

    nc = Bass(trn_type="TRN2", enable_asserts=False, num_swdge_queues=4)

    outputs_l = nc.dram_tensor("outputs_l", [RPC, D], f32, kind="ExternalInput")
    targets_l = nc.dram_tensor("targets_l", [RPC, D], f32, kind="ExternalInput")
    raw_l = nc.dram_tensor("raw_l", [RPC, D], f32, kind="ExternalInput")
    latent = nc.dram_tensor("latent", [B, D], f32, kind="ExternalInput")
    raw = nc.dram_tensor("raw", [B, D], f32, kind="ExternalInput")
    ident_in = nc.dram_tensor("ident", [128, 128], f32, kind="ExternalInput")
    iota_in = nc.dram_tensor("iotac", [128, NCHUNK * 8], u32,
                             kind="ExternalInput")
    # bitmask constants as tensor_tensor operands: cols 0:64 = 0x7FFFF000
    # (embed mask), cols 64:89 = 0xFFF (index decode mask).  DVE
    # tensor_scalar can enter 2-port perf mode and then fully blocks
    # against active SWDGE descriptor generation (the gathers);
    # tensor_tensor never contends, so masks come in as tensors.
    mask_in = nc.dram_tensor("masku", [128, 64 + K], u32,
                             kind="ExternalInput")
    res = nc.dram_tensor("res", [1, 1], f32, kind="ExternalOutput")
    combD = nc.dram_tensor("combD", [B, 2 * D], bf16)
    if dbg:
        dbg_ch = nc.dram_tensor("dbg_ch", [128, SEL_CHUNK], f32,
                                kind="ExternalOutput")
        dbg_cv = nc.dram_tensor("dbg_cv", [128, NCHUNK * 8], f32,
                                kind="ExternalOutput")
        dbg_ci = nc.dram_tensor("dbg_ci", [128, NCHUNK * 8], i32,
                                kind="ExternalOutput")
        dbg_idx = nc.dram_tensor("dbg_idx", [128, K], i32,
                                 kind="ExternalOutput")
        dbg_comb = nc.dram_tensor("dbg_comb", [128, K * 2 * D], f32,
                                  kind="ExternalOutput")
        dbg_stats = nc.dram_tensor("dbg_stats", [128, 8], f32,
                                   kind="ExternalOutput")
        dbg_u = nc.dram_tensor("dbg_u", [128, 2 * D], f32,
                               kind="ExternalOutput")
        dbg_sv = nc.dram_tensor("dbg_sv", [128, K * 2], f32,
                                kind="ExternalOutput")

    A = mybir.AluOpType
    AX = mybir.AxisListType

    def pr(name):
        return reps if phase == name else 1

    with nc.allow_low_precision("bf16/fp16 distance/eig stages within tol"), \
            TileContext(nc) as tc, ExitStack() as ctx:

        const_p = ctx.enter_context(tc.tile_pool(name="const", bufs=1))
        chunk_p = ctx.enter_context(tc.tile_pool(name="chunk", bufs=3))
        sel_p = ctx.enter_context(tc.tile_pool(name="sel", bufs=4))
        eig_p = ctx.enter_context(tc.tile_pool(name="eig", bufs=2))
        cg_p = ctx.enter_context(tc.tile_pool(name="cgp", bufs=4))
        psum_p = ctx.enter_context(tc.tile_pool(name="psum", bufs=4, space="PSUM"))
        psS = ctx.enter_context(tc.tile_pool(name="psS", bufs=2, space="PSUM"))
        cov_p = ctx.enter_context(tc.tile_pool(name="covp", bufs=1, space="PSUM"))

        # ---- constants ----
        ident = const_p.tile([128, 128], f32)
        identb = const_p.tile([128, 128], bf16)
        stats = const_p.tile([128, 8], f32)
        ones64b = const_p.tile([64, 1], bf16)
        ones64f = const_p.tile([64, 1], f32)
        ones128 = const_p.tile([128, 1], f32)
        iota_off = const_p.tile([128, NCHUNK * 8], u32)
        masks = const_p.tile([128, 64 + K], u32)
        negK = const_p.tile([128, 1], f32)
        kepsc = const_p.tile([64, 1], f32)

        # dma_gather (InstDMAGatherAnt) lives in the 'mlp' gpsimd library;
        # iota was replaced with a host-supplied constant so no standard-
        # library op remains and one load at kernel start suffices.
        nc.gpsimd.load_library(library_config.mlp)
        nc.sync.dma_start(ident[:], ident_in[:])
        nc.sync.dma_start(iota_off[:], iota_in[:])
        nc.sync.dma_start(masks[:], mask_in[:])
        nc.vector.tensor_copy(identb[:], ident[:])
        nc.vector.memset(ones64b[:], 1.0)
        nc.vector.memset(ones64f[:], 1.0)
        nc.vector.memset(ones128[:], 1.0)
        nc.vector.memset(stats[:], 0.0)
        nc.vector.memset(negK[:], -1.0 / K)
        nc.vector.memset(kepsc[:], KEPS)
        # Rt[:, g, :] is the [128, 128] f32 stationary R_g with
        # R_g[c, q] = 1 iff c == g*16 + q%16.  matmul(lhsT=R_g, rhs=idxf)
        # replicates rows [16g, 16g+16) of idxf onto all 8 16-partition
        # groups — the dma_gather index buffer wants the 16-partition wrap
        # replicated into every Q7 core window (queue q reads partitions
        # [32q, 32q+32), so full replication covers all queues).
        Rt = const_p.tile([128, 8, 128], f32)
        for g in range(8):
            nc.vector.tensor_copy(
                Rt[:, g, :].rearrange("p (k q) -> p k q", k=8),
                ident[:, 16 * g:16 * (g + 1)].unsqueeze(1)
                .broadcast_to([128, 8, 16]))

        for _rep in range(reps if phase is None else 1):
            # ---- prep: two full-table loads (p-major: 8KB descriptors),
            # bf16 comb table [1 | latent | raw], X matrix ----
            # global row j = 32*p + t  (partition-major layout)
            raw_f = const_p.tile([128, 32, D], f32, tag="rawf")
            lat_f = const_p.tile([128, 32, D], f32, tag="latf")
            comb = const_p.tile([128, 32, 2 * D + 1], bf16, tag="comb")
            nc.sync.dma_start(raw_f[:],
                              raw[:].rearrange("(p t) d -> p t d", p=128))
            nc.sync.dma_start(lat_f[:],
                              latent[:].rearrange("(p t) d -> p t d", p=128))
            # recon + local-raw loads issued early so the DMA overlaps prep
            ob = const_p.tile([128, NT, 64], f32, tag="ob")
            tb = const_p.tile([128, NT, 64], f32, tag="tb")
            nc.sync.dma_start(ob[:],
                              outputs_l[:].rearrange("(p t) d -> p t d", t=NT))
            nc.sync.dma_start(tb[:],
                              targets_l[:].rearrange("(p t) d -> p t d", t=NT))
            rloc = const_p.tile([128, NT, 64], f32, tag="rloc")
            nc.sync.dma_start(rloc[:],
                              raw_l[:].rearrange("(p t) d -> p t d", t=NT))

            # comb layout [raw | latent | 1]: combD = cols 0:128 = [raw|
            # latent] (so the DMA-transposed table puts rawT on partitions
            # 0:64), cov reads the contiguous [latent | 1] = cols 64:129.
            nc.vector.memset(comb[:, :, 2 * D:2 * D + 1], 1.0)
            nc.vector.tensor_copy(comb[:, :, 0:D], raw_f[:])
            nc.vector.tensor_copy(comb[:, :, D:2 * D], lat_f[:])
            # combined bf16 table to DRAM (for the gathers); row j = 32p+t
            nc.sync.dma_start(
                combD[:].rearrange("(p t) c -> p t c", p=128, t=32),
                comb[:, :, 0:2 * D])
            rlocb = const_p.tile([128, NT, 64], bf16, tag="rlocb")
            nc.vector.tensor_copy(rlocb[:], rloc[:])

            # ---- X = [rawT (64 rows); 512-sq] [65, 4096] bf16 ----
            # ONE HWDGE DMA-transpose of combD gives XT = [rawT; latentT];
            # the unused latentT row 0 (partition 64) is overwritten with
            # the bias row 512 - |r_j|^2 (ACT bias folds the +512 embed
            # offset; the DVE tensor_scalar it replaces would block
            # against active SWDGE descriptor generation).
            XT = const_p.tile([128, B], bf16)
            nc.sync.dma_start_transpose(XT[:], combD[:])
            X = XT[0:65, :]
            # Wb stationaries [65, 128]: rows 0:64 = 2*rawT_local (per
            # tile), row 64 = 1 (pairs 512-sq).  The constant row is
            # memset here, before any gather runs.
            Wbs = []
            for t in range(NT):
                Wb = sel_p.tile([65, 128], bf16, tag=f"Wb{t}")
                nc.vector.memset(Wb[64:65, :], 1.0)
                Wbs.append(Wb)
            for c in range(NCHUNK):
                cs = slice(c * SEL_CHUNK, (c + 1) * SEL_CHUNK)
                sq_t = chunk_p.tile([64, SEL_CHUNK], bf16, tag="sqt")
                nc.vector.tensor_mul(sq_t[:], XT[0:64, cs], XT[0:64, cs])
                sq_ps = psS.tile([1, SEL_CHUNK], f32, tag="s", space="PSUM")
                nc.tensor.matmul(out=sq_ps[:], lhsT=ones64b[:], rhs=sq_t[:],
                                 start=True, stop=True)
                # X row 64 = 512 - |r_j|^2 (bias bakes the embed offset in)
                nc.scalar.activation(XT[64:65, cs], sq_ps[:],
                                     mybir.ActivationFunctionType.Copy,
                                     bias=512.0, scale=-1.0)

            # ---- per 128-row tile: selection + batched gather ----
            def emit_selgather(t):
                Wb = Wbs[t]
                rT2_ps = psS.tile([64, 128], bf16, tag="s", space="PSUM")
                nc.tensor.transpose(out=rT2_ps[:], in_=rlocb[:, t, :],
                                    identity=identb[:])
                nc.scalar.mul(Wb[0:64, :], rT2_ps[:], 2.0)

                cand_v = sel_p.tile([128, NCHUNK * 8], f32, tag="cand_v")
                cand_i = sel_p.tile([128, NCHUNK * 8], u32, tag="cand_i")
                for c in range(NCHUNK):
                    ps_d = psum_p.tile([128, SEL_CHUNK], f32, tag="dist",
                                       space="PSUM")
                    for _dr in range(pr("dist")):
                        nc.tensor.matmul(
                            out=ps_d[:], lhsT=Wb[:],
                            rhs=XT[0:65, c * SEL_CHUNK:(c + 1) * SEL_CHUNK],
                            start=True, stop=True)
                    if dbg and t == 0 and c == 0:
                        chf = chunk_p.tile([128, SEL_CHUNK], f32, tag="chf")
                        nc.vector.tensor_copy(chf[:], ps_d[:])
                        nc.sync.dma_start(dbg_ch[:], chf[:])
                    for _sr in range(pr("sel")):
                        nc.vector.max(cand_v[:, c * 8:(c + 1) * 8], ps_d[:])
                        nc.vector.max_index(cand_i[:, c * 8:(c + 1) * 8],
                                            cand_v[:, c * 8:(c + 1) * 8],
                                            ps_d[:])
                # embed global index into low 12 mantissa bits of the 64
                # candidates (+512 bias first: values ~[350,620),
                # quantum <= 0.25)
                emb = sel_p.tile([128, NCHUNK * 8], u32, tag="emb")
                top32 = sel_p.tile([128, 32], u32, tag="top32")
                idx32 = sel_p.tile([128, K], i32, tag="idx32")
                for _sr in range(pr("sel")):
                    nc.vector.tensor_tensor(out=cand_i[:], in0=cand_i[:],
                                            in1=iota_off[:], op=A.add)
                    # cand_v already carries the +512 bias from the matmul
                    nc.vector.tensor_tensor(out=emb[:],
                                            in0=cand_v[:].bitcast(u32),
                                            in1=masks[:, 0:64],
                                            op=A.bitwise_and)
                    nc.vector.tensor_tensor(out=emb[:], in0=emb[:],
                                            in1=cand_i[:], op=A.bitwise_or)
                    for r in range(4):
                        nc.vector.max(top32[:, r * 8:(r + 1) * 8].bitcast(f32),
                                      emb[:].bitcast(f32))
                        if r < 3:
                            nc.vector.match_replace(
                                out=emb[:].bitcast(f32),
                                in_to_replace=top32[:, r * 8:(r + 1) * 8]
                                .bitcast(f32),
                                in_values=emb[:].bitcast(f32), imm_value=0.0)
                    # decode 25 neighbor indices (drop rank 0 = self)
                    nc.vector.tensor_tensor(out=idx32[:].bitcast(u32),
                                            in0=top32[:, 1:1 + K],
                                            in1=masks[:, 64:64 + K],
                                            op=A.bitwise_and)

                if dbg and t == 0:
                    nc.sync.dma_start(dbg_cv[:], cand_v[:])
                    nc.sync.dma_start(dbg_ci[:], cand_i[:].bitcast(i32))
                    nc.sync.dma_start(dbg_idx[:], idx32[:])
                comb_g = cg_p.tile([128, K, 2 * D], bf16, tag="comb_g")
                if ablate == "gather":
                    nc.vector.memset(comb_g[:], 1.0)
                else:
                    # ONE batched dma_gather per tile.  dma_gather reads
                    # int16 indices wrapped in 16 partitions (linear
                    # n = s*16 + p), replicated to every 16-partition
                    # group, and writes gathered row n to
                    # dst[n % 128, n // 128, :]: with n = a*128 + i the
                    # index at [p, 8a + g] must be idx32[g*16 + p, a].
                    # E16 matmuls replicate rows 16g..16g+16 across all
                    # partition groups; the strided cast-copy interleaves
                    # (g, a) -> column a*8 + g.
                    # both casts run on the Scalar engine so the gather's
                    # input chain never sits behind eig work in the
                    # in-order Vector queue.
                    idxf = sel_p.tile([128, K], f32, tag="idxf")
                    nc.scalar.copy(idxf[:], idx32[:])
                    rep_ps = psS.tile([128, 8, K], f32, tag="s", space="PSUM")
                    for g in range(8):
                        nc.tensor.matmul(out=rep_ps[:, g, :],
                                         lhsT=Rt[:, g, :], rhs=idxf[:],
                                         start=True, stop=True)
                    idxs16 = sel_p.tile([128, 8 * K], i16, tag="idxs16")
                    nc.scalar.copy(
                        idxs16[:].rearrange("p (a g) -> p g a", g=8),
                        rep_ps[:])
                    for _gr in range(pr("gather")):
                        # single_packet=False: coalescing 3200 descs into
                        # one packet per engine exceeds the <=64-descriptor
                        # packet ceiling and wedges the device.  One SWDGE
                        # queue per tile: descriptor generation runs on a
                        # different Q7 core pair per queue and overlaps.
                        nc.gpsimd.dma_gather(
                            out_ap=comb_g[:], in_ap=combD[:],
                            idxs_ap=idxs16[:], num_idxs=128 * K,
                            num_idxs_reg=128 * K, elem_size=2 * D,
                            single_packet=False, queue_num=t)

                if dbg and t == 0:
                    cgf = eig_p.tile([128, K * 2 * D], f32, tag="cgf")
                    nc.vector.tensor_copy(
                        cgf[:], comb_g[:].rearrange("p k c -> p (k c)"))
                    nc.sync.dma_start(dbg_comb[:], cgf[:])
                return comb_g, idx32

            # ---- eig: both sides jointly; one power iteration ----
            eig_count = [0]

            def emit_eig(comb_g, gate_b):
                first_eig = eig_count[0] == 0
                eig_count[0] += 1
                for _er in range(pr("eig")):
                    # gate_b is all-zeros, produced after the LAST tile's
                    # index decode: a real dependency on the FIRST op of
                    # the eig chain keeps every eig Vector op behind all
                    # selection/decode Vector ops in any schedule (the
                    # in-order Vector queue would otherwise head-of-line
                    # block later selections — and thus the gathers — on
                    # this eig's gather data).  All other eig ops depend
                    # on v0 or its descendants.
                    v0 = eig_p.tile([128, 2 * D], bf16, tag="v0")
                    nc.vector.tensor_tensor(
                        out=v0[:], in0=comb_g[:, 0, :],
                        in1=gate_b[:].broadcast_to([128, 2 * D]), op=A.add)
                    nc.vector.tensor_sub(v0[:], v0[:], comb_g[:, 1, :])
                    t1 = eig_p.tile([128, K, 2 * D], bf16, tag="t1")
                    nc.vector.tensor_tensor(
                        out=t1[:], in0=comb_g[:],
                        in1=v0[:].unsqueeze(1).broadcast_to([128, K, 2 * D]),
                        op=A.mult)
                    # per-(neighbor, side) dots: reduce innermost 64
                    s_v = eig_p.tile([128, K, 2], f32, tag="sv")
                    nc.vector.tensor_reduce(
                        out=s_v[:],
                        in_=t1[:].rearrange("p k (s d) -> p k s d", s=2),
                        axis=AX.X, op=A.add)
                    ssum = eig_p.tile([128, 2], f32, tag="ssum")
                    nc.vector.tensor_reduce(
                        out=ssum[:], in_=s_v[:].rearrange("p k s -> p s k"),
                        axis=AX.X, op=A.add)
                    # center: s = s - mean_k(s), via two tensor_tensor ops
                    # (scalar_tensor_tensor may enter the 2-port perf mode
                    # that blocks against active SWDGE generation)
                    nc.vector.tensor_tensor(
                        out=ssum[:], in0=ssum[:],
                        in1=negK[:].broadcast_to([128, 2]), op=A.mult)
                    nc.vector.tensor_tensor(
                        out=s_v[:],
                        in0=ssum[:].unsqueeze(1).broadcast_to([128, K, 2]),
                        in1=s_v[:], op=A.add)
                    # t2 = Y * s_bc, then tree-reduce over k
                    t2 = eig_p.tile([128, K + 7, 2 * D], bf16, tag="t2")
                    nc.vector.tensor_tensor(
                        out=t2[:, 0:K, :].rearrange("p k (s d) -> p k s d", s=2),
                        in0=comb_g[:].rearrange("p k (s d) -> p k s d", s=2),
                        in1=s_v[:].unsqueeze(3).broadcast_to([128, K, 2, D]),
                        op=A.mult)
                    n = K
                    while n > 1:
                        h = n // 2
                        nc.vector.tensor_add(t2[:, 0:h, :], t2[:, 0:h, :],
                                             t2[:, h:2 * h, :])
                        if n % 2:
                            # move via add-zero: tensor_copy can enter the
                            # 2-port mode that blocks against SWDGE
                            nc.vector.tensor_tensor(
                                out=t2[:, h:h + 1, :],
                                in0=t2[:, n - 1:n, :],
                                in1=gate_b[:].unsqueeze(1)
                                .broadcast_to([128, 1, 2 * D]), op=A.add)
                            n = h + 1
                        else:
                            n = h
                    if dbg and first_eig:
                        svf = eig_p.tile([128, K * 2], f32, tag="svf")
                        nc.vector.tensor_copy(
                            svf[:], s_v[:].rearrange("p k s -> p (k s)"))
                        nc.sync.dma_start(dbg_sv[:], svf[:])
                    # overlap stats: q = (uz.ux)^2 / (|uz|^2 |ux|^2)
                    u = t2[:, 0, :]
                    u2 = eig_p.tile([128, 2 * D], f32, tag="u2")
                    nc.vector.tensor_mul(u2[:], u, u)
                    nn_v = eig_p.tile([128, 2], f32, tag="nn")
                    nc.vector.tensor_reduce(
                        out=nn_v[:], in_=u2[:].rearrange("p (s d) -> p s d", s=2),
                        axis=AX.X, op=A.add)
                    cr = eig_p.tile([128, D], f32, tag="cr")
                    nc.vector.tensor_mul(cr[:], u[:, 0:D], u[:, D:2 * D])
                    dzx = eig_p.tile([128, 2], f32, tag="dzx")
                    nc.vector.tensor_reduce(out=dzx[:, 0:1], in_=cr[:],
                                            axis=AX.X, op=A.add)
                    nc.vector.tensor_mul(dzx[:, 1:2], nn_v[:, 0:1], nn_v[:, 1:2])
                    # +1 guards div-by-zero for degenerate rows (duplicate
                    # neighbors from exact distance ties); den is ~1e10
                    # normally so the bias is negligible.
                    nc.vector.tensor_add(dzx[:, 1:2], dzx[:, 1:2],
                                         ones128[:, 0:1])
                    nc.vector.reciprocal(dzx[:, 1:2], dzx[:, 1:2])
                    nc.vector.tensor_mul(dzx[:, 0:1], dzx[:, 0:1], dzx[:, 0:1])
                    nc.vector.tensor_mul(dzx[:, 0:1], dzx[:, 0:1], dzx[:, 1:2])
                    nc.vector.tensor_add(stats[:, 1:2], stats[:, 1:2],
                                         dzx[:, 0:1])
                    if dbg and first_eig:
                        uf = eig_p.tile([128, 2 * D], f32, tag="uf")
                        nc.vector.tensor_copy(uf[:], u)
                        nc.sync.dma_start(dbg_u[:], uf[:])

            # all 4 selections+gathers first: the gathers start as early as
            # possible and run on their own SWDGE queues while the Vector
            # engine keeps busy with the following selections + cov/recon.
            sel_out = [emit_selgather(t) for t in range(NT)]
            cgs = [cg for cg, _ in sel_out]
            # zero gate derived from the LAST tile's decoded indices
            # (x - x == 0; tensor_tensor form never contends with SWDGE)
            gate_b = const_p.tile([128, 1], bf16, tag="gate")
            nc.vector.tensor_tensor(out=gate_b[:], in0=sel_out[-1][1][:, 0:1],
                                    in1=sel_out[-1][1][:, 0:1],
                                    op=A.subtract)

            # ---- global latent covariance on PE (bf16), fused with the
            # column-sum via the ones column: out [64, 65], cols 0:64 =
            # cov, col 64 = s ----
            cov_ps = cov_p.tile([64, 65], f32, space="PSUM")
            for t in range(32):
                nc.tensor.matmul(out=cov_ps[:], lhsT=comb[:, t, D:2 * D],
                                 rhs=comb[:, t, D:2 * D + 1],
                                 start=(t == 0), stop=(t == 31))

            # ---- cov postprocessing: C, trC, trC2, lambda_max ----
            cov_s = const_p.tile([64, 65], f32, tag="covs")
            nc.scalar.copy(cov_s[:], cov_ps[:])
            # s as a row: s_row[0, f] = s[f] via lhsT = s_col (col 64)
            srow_ps = psS.tile([1, 64], f32, tag="s", space="PSUM")
            nc.tensor.matmul(out=srow_ps[:], lhsT=cov_s[:, 64:65],
                             rhs=ident[0:64, 0:64], start=True, stop=True)
            s_row = const_p.tile([1, 64], f32, tag="srow")
            nc.scalar.copy(s_row[:], srow_ps[:])
            ssT_ps = psS.tile([64, 64], f32, tag="s", space="PSUM")
            nc.tensor.matmul(out=ssT_ps[:], lhsT=s_row[:], rhs=s_row[:],
                             start=True, stop=True)
            sst_s = const_p.tile([64, 64], f32, tag="sst")
            nc.scalar.mul(sst_s[:], ssT_ps[:], KEPS / B)
            C_s = const_p.tile([64, 64], f32, tag="Cs")
            # C = cov*KEPS - ssT*(KEPS/B), via tensor_tensor forms only
            # (this block overlaps the gathers)
            nc.vector.tensor_tensor(out=C_s[:], in0=cov_s[:, 0:D],
                                    in1=kepsc[:].broadcast_to([64, 64]),
                                    op=A.mult)
            nc.vector.tensor_sub(C_s[:], C_s[:], sst_s[:])
            diag_scr = const_p.tile([64, 64], f32, tag="dscr")
            nc.vector.tensor_mul(diag_scr[:], C_s[:], ident[0:64, 0:64])
            nc.vector.tensor_reduce(out=stats[0:64, 2:3], in_=diag_scr[:],
                                    axis=AX.X, op=A.add)
            fro_scr = const_p.tile([64, 64], f32, tag="fscr")
            nc.vector.tensor_mul(fro_scr[:], C_s[:], C_s[:])
            nc.vector.tensor_reduce(out=stats[0:64, 3:4], in_=fro_scr[:],
                                    axis=AX.X, op=A.add)
            # 5 squarings: M = C^32, then Rayleigh via w = M.1
            M_prev = C_s
            for sqi in range(5):
                m_ps = psS.tile([64, 64], f32, tag="s", space="PSUM")
                nc.tensor.matmul(out=m_ps[:], lhsT=M_prev[:], rhs=M_prev[:],
                                 start=True, stop=True)
                M_new = const_p.tile([64, 64], f32, tag=f"m{sqi}")
                nc.scalar.copy(M_new[:], m_ps[:])
                M_prev = M_new
            w_ps = psS.tile([64, 1], f32, tag="s", space="PSUM")
            nc.tensor.matmul(out=w_ps[:], lhsT=M_prev[:], rhs=ones64f[:],
                             start=True, stop=True)
            w_s = const_p.tile([64, 1], f32, tag="ws")
            nc.scalar.copy(w_s[:], w_ps[:])
            r_ps = psS.tile([64, 1], f32, tag="s", space="PSUM")
            nc.tensor.matmul(out=r_ps[:], lhsT=C_s[:], rhs=w_s[:],
                             start=True, stop=True)
            nc.vector.tensor_mul(stats[0:64, 4:5], w_s[:], r_ps[:])
            nc.vector.tensor_mul(stats[0:64, 5:6], w_s[:], w_s[:])

            # ---- recon over this core's 512-row slice ----
            dif = const_p.tile([128, NT, 64], f32, tag="dif")
            nc.vector.tensor_sub(dif[:], ob[:], tb[:])
            nc.vector.tensor_mul(dif[:], dif[:], dif[:])
            nc.vector.tensor_reduce(out=stats[:, 0:1], in_=dif[:],
                                    axis=AX.XY, op=A.add)

            # ---- eig stages, in gather-completion order ----
            if ablate != "eig":
                for t in range(NT):
                    emit_eig(cgs[t], gate_b)

        if dbg:
            nc.sync.dma_start(dbg_stats[:], stats[:])
        # ---- final scalar assembly ----
        fin_ps = psS.tile([1, 8], f32, tag="s", space="PSUM")
        nc.tensor.matmul(out=fin_ps[:], lhsT=ones128[:], rhs=stats[:],
                         start=True, stop=True)
        fin = const_p.tile([1, 8], f32, tag="fin")
        nc.scalar.copy(fin[:], fin_ps[:])
        sc = const_p.tile([1, 8], f32, tag="sc")
        res_s = const_p.tile([1, 1], f32, tag="ress")
        nc.vector.reciprocal(sc[:, 0:1], fin[:, 3:4])          # 1/trC2
        nc.vector.reciprocal(sc[:, 1:2], fin[:, 5:6])          # 1/(w.w)
        nc.vector.reciprocal(sc[:, 2:3], fin[:, 2:3])          # 1/trC
        nc.vector.tensor_mul(sc[:, 3:4], fin[:, 2:3], fin[:, 2:3])
        nc.vector.tensor_mul(sc[:, 3:4], sc[:, 3:4], sc[:, 0:1])   # pr ratio
        nc.vector.tensor_mul(sc[:, 4:5], fin[:, 4:5], sc[:, 1:2])  # lambda
        nc.vector.tensor_mul(sc[:, 4:5], sc[:, 4:5], sc[:, 2:3])   # lam/trC
        # S = f0/262144 + 0.02625 - (0.2/4096) f1 + 0.00125 pr - 0.00125 q
        nc.vector.tensor_scalar(res_s[:], fin[:, 0:1], 1.0 / (B * D), 0.02625,
                                op0=A.mult, op1=A.add)
        nc.vector.scalar_tensor_tensor(out=res_s[:], in0=fin[:, 1:2],
                                       scalar=-0.2 / B, in1=res_s[:],
                                       op0=A.mult, op1=A.add)
        nc.vector.scalar_tensor_tensor(out=res_s[:], in0=sc[:, 3:4],
                                       scalar=0.00125, in1=res_s[:],
                                       op0=A.mult, op1=A.add)
        nc.vector.scalar_tensor_tensor(out=res_s[:], in0=sc[:, 4:5],
                                       scalar=-0.00125, in1=res_s[:],
                                       op0=A.mult, op1=A.add)
        nc.sync.dma_start(res[:], res_s[:])

    # Raw Bass skips Bacc.compile(); fill in the ISA encoding bytes for
    # extended-inst ISA subclasses (PseudoReloadLibraryIndex) — walrus
    # rejects empty .instr with "ISA wrong length".
    mybir.codegen_inst_isa_subclasses(nc)
    return nc


def get_nc(reps=1, phase=None, ablate=None, dbg=False):
    key = ("nc", reps, phase, ablate, dbg)
    if key not in _CACHE:
        _CACHE[key] = _build_bass(reps, phase, ablate, dbg)
    return _CACHE[key]


def make_in_maps(inputs):
    ident = np.eye(128, dtype=np.float32)
    iotac = np.broadcast_to(
        (np.arange(NCHUNK, dtype=np.uint32) * SEL_CHUNK)
        .repeat(8)[None, :], (128, NCHUNK * 8)).copy()
    masku = np.broadcast_to(
        np.concatenate([np.full(64, 0x7FFFF000, np.uint32),
                        np.full(K, 0x00000FFF, np.uint32)])[None, :],
        (128, 64 + K)).copy()
    outs = np.ascontiguousarray(inputs["outputs"], np.float32)
    tgts = np.ascontiguousarray(inputs["targets"], np.float32)
    lat = np.ascontiguousarray(inputs["latent"], np.float32)
    rawf = np.ascontiguousarray(inputs["raw"], np.float32)
    maps = []
    for c in range(NCORES):
        sl = slice(c * RPC, (c + 1) * RPC)
        maps.append({
            "outputs_l": np.ascontiguousarray(outs[sl]),
            "targets_l": np.ascontiguousarray(tgts[sl]),
            "raw_l": np.ascontiguousarray(rawf[sl]),
            "latent": lat,
            "raw": rawf,
            "ident": ident,
            "iotac": iotac,
            "masku": masku,
        })
    return maps


def kernel(**inputs) -> np.ndarray:
    os.environ.setdefault("JAX_PLATFORMS", "")
    from concourse.bass_utils import run_bass_kernel_spmd

    nc = get_nc()
    in_maps = make_in_maps(inputs)
    r = run_bass_kernel_spmd(nc, in_maps, core_ids=list(range(NCORES)))
    total = np.float32(0.0)
    for dev in r.results:
        total = np.float32(total + np.float32(dev["res"].reshape(())))
    return np.asarray(total, dtype=np.float32)


if __name__ == "__main__":
    nc = get_nc()
    print("bass build OK:", nc)
